# revision 36
# baseline (speedup 1.0000x reference)
"""Trainium2 Bass kernel for nn_ContextualLoss (8 NeuronCores, SPMD).

Math (derived from the reference):
  X = vec(input)[10:50] - mu,  T = vec(target)[10:50] - mu,  mu = colmean(target)
  S[i,j] = cos(x_i, t_j);  CX = softmax_j(a_i * S[i,j]),  a_i = -1/(h*(min_j S + eps))
  loss = -log(max_ij CX)
X's column normalization folds into the softmax temperature:
  logits = b_i * R[i,j],  R = Xc^T Tn  (Xc centered only, Tn column-normalized)
  b_i = -1/(h*(min_j R + eps*||x_i||))
Per row we need: min_j R (f32/fp16), max_j R (fp16), sum_j exp(b_i R) (f32 acc).
Final: per-core max of exp(b*M)/Z, AllReduce-max over 8 cores, -log.

Sharding: each core computes 2048 of the 16384 S-rows (x-columns).
"""

import numpy as np
from contextlib import ExitStack

import concourse.bass as bass
import concourse.mybir as mybir
from concourse import bass_isa

F32 = mybir.dt.float32
F16 = mybir.dt.float16
AF = mybir.ActivationFunctionType
OP = mybir.AluOpType
AX = mybir.AxisListType

D = 40          # contraction dim (rows 10:50)
N = 16384       # feature columns
P = 8           # cores
SH = N // P     # x-columns per core
NRB = SH // 128  # row blocks per core = 16
NG = 8          # 2048-wide column groups per row block
NGA = 5         # groups extracted by ScalarE (plain copy)
GD = [5, 6, 7]  # groups extracted by VectorE (TTR copy+min)
GRP = 2048      # group width
EPS = 1e-5
H = 0.2
BIGF = 3.0e38


def _tree_thunks(v, src, width, op, out_slice, TA, TB, stop=512):
    """Thunks for a pairwise-halving fp16 reduce on VectorE.

    Returned thunks must be interleaved with >=1 unrelated op between
    consecutive ones (DVE pipeline output hazard: an op may not read the
    immediately-preceding op's output without a drain)."""
    thunks = []
    cur, w = src, width
    bufs = [TA, TB]
    bi = 0
    while w > stop:
        h = w // 2
        dst = bufs[bi]
        thunks.append(lambda cur=cur, h=h, w=w, dst=dst:
                      v.tensor_tensor(dst[:, 0:h], cur[:, 0:h], cur[:, h:w],
                                      op=op))
        cur, w = bufs[bi], h
        bi ^= 1
    thunks.append(lambda cur=cur, w=w:
                  v.tensor_reduce(out_slice, cur[:, 0:w], axis=AX.X, op=op))
    return thunks


def _interleave(v, *lists):
    """Emit ops round-robin from the lists with a drain between rounds.

    The race model requires every same-engine RAW/WAW pair to be separated
    by a drain (or a sem-certified wait); ops within one round are mutually
    independent, so one drain per round suffices and overlaps round issue."""
    n = max(len(l) for l in lists)
    for i in range(n):
        for l in lists:
            if i < len(l):
                l[i]()
        if i < n - 1:
            v.drain()


def build():
    import os
    stage = os.environ.get("K_STAGE", "full")   # debug bisect hook
    if stage == "pre":
        NB = 0
    elif stage.startswith("main"):
        NB = int(stage[4:])
    else:
        NB = NRB

    nc = bass.Bass(num_devices=P)

    xs_d = nc.declare_dram_parameter("xs", [D, SH], F32, isOutput=False)
    ts_d = nc.declare_dram_parameter("ts", [D, SH], F32, isOutput=False)
    t_d = nc.declare_dram_parameter("t", [D, N], F32, isOutput=False)
    out_d = nc.declare_dram_parameter("out", [1, 1], F32, isOutput=True)

    eye40_d = nc.inline_tensor(np.eye(D, dtype=np.float32), "eye40c")
    eye128_d = nc.inline_tensor(np.eye(128, dtype=np.float16), "eye128c")
    cc_in = nc.dram_tensor("cc_in", [1, 1], F32)
    cc_out = nc.dram_tensor("cc_out", [1, 1], F32, addr_space="Shared")

    ctx = ExitStack()
    with ctx:
        sbuf = lambda name, shape, dt: ctx.enter_context(
            nc.sbuf_tensor(name, shape, dt))
        sem = lambda name: ctx.enter_context(nc.semaphore(name))

        # ---- persistent SBUF ----
        eye40 = sbuf("eye40", [D, D], F32)
        eye128 = sbuf("eye128", [128, 128], F16)
        Tn = sbuf("Tn", [D, N], F16)         # normalized target, matmul rhs
        Xs = sbuf("Xsb", [D, SH], F16)       # centered x shard, matmul lhsT
        zeros = sbuf("zeros", [128, GRP], F32)
        epsnx = sbuf("epsnx", [128, NRB], F32)
        Mall = sbuf("Mall", [128, NRB], F32)
        mall = sbuf("mall", [128, NRB], F32)
        ball = sbuf("ball", [128, NRB], F32)
        Zall = sbuf("Zall", [128, NRB], F32)
        r_all = sbuf("r_all", [128, NRB], F32)
        minc = sbuf("minc", [128, len(GD)], F32)
        Upar = [sbuf(f"U{i}", [128, 2], F32) for i in range(2)]
        Epar = [sbuf(f"E{i}", [128, 2], F32) for i in range(2)]
        Zpar = [sbuf(f"Zp{i}", [128, 4], F32) for i in range(2)]
        mtree = sbuf("mtree", [128, 1], F32)
        mc1 = sbuf("mc1", [128, 1], F32)
        mc2 = sbuf("mc2", [128, 1], F32)
        mp = sbuf("mp", [128, 1], F32)
        dd = sbuf("dd", [128, 1], F32)
        zr = sbuf("zr", [128, 1], F32)
        wm = sbuf("wm", [128, 1], F32)
        rmaxb = sbuf("rmaxb", [128, 1], F32)
        prb = sbuf("prb", [128, 1], F32)
        gmb = sbuf("gmb", [1, 1], F32)
        lnb = sbuf("lnb", [1, 1], F32)
        outsb = sbuf("outsb", [1, 1], F32)

        s_dma = sem("s_dma")
        s_e40 = sem("s_e40")
        s_e128 = sem("s_e128")
        s_xs = sem("s_xs")
        s_ts = sem("s_ts")
        s_tc = [sem("s_tc0"), sem("s_tc1")]
        s_tp = sem("s_tp")
        s_tpc = sem("s_tpc")
        s_stat = sem("s_stat")
        s_sqrt = sem("s_sqrt")
        s_tno = sem("s_tno")
        s_tnv = sem("s_tnv")
        s_tna = sem("s_tna")
        s_mm = sem("s_mm")
        s_ev = sem("s_ev")
        s_ea = sem("s_ea")
        s_b = sem("s_b")
        s_z = sem("s_z")
        s_r = sem("s_r")
        s_fin = sem("s_fin")
        s_cdma = sem("s_cdma")

        # outward-copy engine assignment: og 0..17 (16 Tn groups + 2 Xs groups)
        out_eng = ["v" if (og % 8) < 5 else "a" for og in range(16)] + ["v", "v"]
        # cumulative per-engine outward-copy counts after og
        ocv, oca = [], []
        cv = ca = 0
        for og in range(18):
            if out_eng[og] == "v":
                cv += 1
            else:
                ca += 1
            ocv.append(cv)
            oca.append(ca)
        TOT_V, TOT_A = cv, ca

        # main-loop extraction engine per group index (within rb)
        def ext_is_act(g):
            return g < NGA

        # cumulative extraction counts after global group K
        act_cum, dve_cum = [], []
        a = v = 0
        for K in range(NRB * NG):
            if ext_is_act(K % NG):
                a += 1
            else:
                v += 1
            act_cum.append(a)
            dve_cum.append(v)

        with nc.Block() as block:
            # ================= PHASE 1: load + preprocess =================
            with ExitStack() as p1:
                sb1 = lambda name, shape, dt: p1.enter_context(
                    nc.sbuf_tensor(name, shape, dt))
                Traw0 = sb1("Traw0", [D, 4096], F32)
                Traw1 = sb1("Traw1", [D, 4096], F32)
                Xraw = sb1("Xraw", [D, SH], F32)
                Tsraw = sb1("Tsraw", [D, SH], F32)
                Tt = sb1("Tt", [128, 5120], F32)
                Tc = sb1("Tc", [128, 5120], F32)
                sqb = sb1("sqb", [128, 5120], F32)
                Tnt = sb1("Tnt", [128, 5120], F16)
                Xt = sb1("Xt", [128, 640], F32)
                Xct = sb1("Xct", [128, 640], F32)
                xsqb = sb1("xsqb", [128, 640], F32)
                Xct16 = sb1("Xct16", [128, 640], F16)
                Tst = sb1("Tst", [128, 640], F32)
                muS = sb1("muS", [128, 128], F32)
                mub = sb1("mub", [128, 128], F32)
                n2S = sb1("n2S", [128, 128], F32)
                n2c = sb1("n2c", [128, 128], F32)
                normb = sb1("normb", [128, 128], F32)
                y0b = sb1("y0b", [128, 128], F32)
                t1b = sb1("t1b", [128, 128], F32)
                t2b = sb1("t2b", [128, 128], F32)
                t3b = sb1("t3b", [128, 128], F32)
                rTb = sb1("rTb", [128, 128], F32)
                xmuS = sb1("xmuS", [128, NRB], F32)
                xmub = sb1("xmub", [128, NRB], F32)
                xn2S = sb1("xn2S", [128, NRB], F32)
                xn2c = sb1("xn2c", [128, NRB], F32)
                xnormb = sb1("xnormb", [128, NRB], F32)
                psT = p1.enter_context(nc.psum_tensor("psT", [128, 2048], F32))
                ps16 = p1.enter_context(nc.psum_tensor("ps16", [128, 4096], F16))
                Traws = [Traw0, Traw1]
                Tt3 = Tt[:, :].rearrange("p (t d) -> p t d", d=D)
                Tc3 = Tc[:, :].rearrange("p (t d) -> p t d", d=D)
                sq3 = sqb[:, :].rearrange("p (t d) -> p t d", d=D)
                Tnt3 = Tnt[:, :].rearrange("p (t d) -> p t d", d=D)
                Xt3 = Xt[:, :].rearrange("p (t d) -> p t d", d=D)
                Xct3 = Xct[:, :].rearrange("p (t d) -> p t d", d=D)
                xsq3 = xsqb[:, :].rearrange("p (t d) -> p t d", d=D)
                Tst3 = Tst[:, :].rearrange("p (t d) -> p t d", d=D)

                @block.sync
                def _(sy):
                    sy.dma_start(out=eye40[:, :], in_=eye40_d[:, :]).then_inc(s_e40, 16)
                    sy.dma_start(out=eye128[:, :], in_=eye128_d[:, :]).then_inc(s_e128, 16)
                    sy.dma_start(out=Xraw[:, :], in_=xs_d[:, :]).then_inc(s_xs, 16)
                    sy.dma_start(out=Tsraw[:, :], in_=ts_d[:, :]).then_inc(s_ts, 16)
                    for ci in range(4):
                        if ci >= 2:
                            # buffer reuse: transposes of chunk ci-2 done
                            sy.wait_ge(s_tp, 4 + 4 * (ci - 1))
                        sy.dma_start(
                            out=Traws[ci % 2][:, :],
                            in_=t_d[:, ci * 4096:(ci + 1) * 4096],
                        ).then_inc(s_tc[ci % 2], 16)

                @block.tensor
                def _(pe):
                    # inward transposes: [40,128] f32 tiles -> [128,40] in psT
                    # groups of 8 tiles; psT bank = grp % 4
                    def inw(grp, src_ap_fn):
                        if grp >= 4:
                            pe.wait_ge(s_tpc, grp - 3)
                        for k in range(8):
                            ins = pe.transpose(
                                psT[:, (grp % 4) * 512 + k * D:
                                    (grp % 4) * 512 + (k + 1) * D],
                                src_ap_fn(k),
                                eye40[:, :],
                            )
                        ins.then_inc(s_tp)

                    pe.wait_ge(s_e40, 16)
                    pe.wait_ge(s_e128, 16)
                    pe.wait_ge(s_xs, 16)
                    for g in range(2):
                        inw(g, lambda k, g=g: Xraw[:, (g * 8 + k) * 128:
                                                   (g * 8 + k + 1) * 128])
                    pe.wait_ge(s_ts, 16)
                    for g in range(2):
                        inw(2 + g, lambda k, g=g: Tsraw[:, (g * 8 + k) * 128:
                                                        (g * 8 + k + 1) * 128])
                    for ci in range(4):
                        pe.wait_ge(s_tc[ci % 2], 16 * (ci // 2 + 1))
                        for j in range(4):
                            inw(4 + ci * 4 + j,
                                lambda k, j=j, ci=ci: Traws[ci % 2][
                                    :, (j * 8 + k) * 128:(j * 8 + k + 1) * 128])

                    # outward transposes: [128,40] f16 tiles -> [40,128] in ps16
                    def outw(og, src3, base_tile):
                        if og >= 4:
                            eng = out_eng[og - 4]
                            pe.wait_ge(s_tnv if eng == "v" else s_tna,
                                       ocv[og - 4] if eng == "v" else oca[og - 4])
                        for k in range(8):
                            t = base_tile + k
                            ins = pe.transpose(
                                ps16[0:D, (og % 4) * 1024 + k * 128:
                                     (og % 4) * 1024 + (k + 1) * 128],
                                src3[:, t * D:(t + 1) * D],
                                eye128[:, :],
                            )
                        ins.then_inc(s_tno)

                    pe.wait_ge(s_stat, 4)      # Tnt ready
                    for og in range(16):
                        outw(og, Tnt[:, :], og * 8)
                    pe.wait_ge(s_stat, 3)      # Xct16 ready (covered by 4)
                    for og in range(16, 18):
                        outw(og, Xct16[:, :], (og - 16) * 8)

                @block.vector
                def _(v):
                    v.memset(zeros[:, :], 0.0)
                    v.drain()
                    # inward copies psT bank -> transposed layouts
                    for g in range(20):
                        v.wait_ge(s_tp, g + 1)
                        if g < 2:
                            dest = Xt[:, g * 320:(g + 1) * 320]
                        elif g < 4:
                            dest = Tst[:, (g - 2) * 320:(g - 1) * 320]
                        else:
                            dest = Tt[:, (g - 4) * 320:(g - 3) * 320]
                        v.tensor_copy(dest, psT[:, (g % 4) * 512:(g % 4) * 512 + 320]
                                      ).then_inc(s_tpc)
                    # stats: T-chain and X-chain alternate so no op reads the
                    # immediately-preceding op's output (DVE pipeline hazard)
                    v.wait_ge(s_tpc, 20)
                    mu_bc = mub[:, :].unsqueeze(2).broadcast_to((128, 128, D))
                    xmu_bc = xmub[:, :].unsqueeze(2).broadcast_to((128, NRB, D))
                    tchain = [
                        lambda: v.reduce_sum(muS[:, :], Tt3, axis=AX.X),
                        lambda: v.tensor_scalar_mul(mub[:, :], muS[:, :], 1.0 / D),
                        lambda: v.tensor_sub(Tc3, Tt3, mu_bc),
                        lambda: v.tensor_mul(sq3, Tc3, Tc3),
                        lambda: v.reduce_sum(n2S[:, :], sq3, axis=AX.X),
                        lambda: v.tensor_scalar_max(n2c[:, :], n2S[:, :], 1e-16
                                                    ).then_inc(s_stat),  # -> 1
                    ]
                    xchain = [
                        lambda: v.reduce_sum(xmuS[:, :], Tst3, axis=AX.X),
                        lambda: v.tensor_scalar_mul(xmub[:, :], xmuS[:, :],
                                                    1.0 / D),
                        lambda: v.tensor_sub(Xct3, Xt3, xmu_bc),
                        lambda: v.tensor_mul(xsq3, Xct3, Xct3),
                        lambda: v.reduce_sum(xn2S[:, :], xsq3, axis=AX.X),
                        lambda: v.tensor_scalar_max(xn2c[:, :], xn2S[:, :],
                                                    1e-16).then_inc(s_stat),  # 2
                    ]
                    _interleave(v, tchain, xchain)
                    # rsqrt refine (needs ACT sqrt of n2c -> normb)
                    v.wait_ge(s_sqrt, 1)
                    r_bc = rTb[:, :].unsqueeze(2).broadcast_to((128, 128, D))
                    nrchain = [
                        lambda: v.reciprocal(y0b[:, :], normb[:, :]),
                        lambda: v.tensor_mul(t1b[:, :], y0b[:, :], y0b[:, :]),
                        lambda: v.tensor_mul(t2b[:, :], t1b[:, :], n2c[:, :]),
                        lambda: v.tensor_scalar(t3b[:, :], t2b[:, :], -0.5, 1.5,
                                                op0=OP.mult, op1=OP.add),
                        lambda: v.tensor_mul(rTb[:, :], y0b[:, :], t3b[:, :]),
                        lambda: v.tensor_mul(Tnt3, Tc3, r_bc
                                             ).then_inc(s_stat),   # -> 4
                    ]
                    othchain = [
                        lambda: v.tensor_scalar_mul(epsnx[:, :], xnormb[:, :],
                                                    EPS),
                        lambda: v.tensor_copy(Xct16[:, :], Xct[:, :]
                                              ).then_inc(s_stat),  # -> 3
                    ]
                    _interleave(v, nrchain, othchain)
                    # outward copies (vector share)
                    for og in range(18):
                        if out_eng[og] != "v":
                            continue
                        v.wait_ge(s_tno, og + 1)
                        src = ps16[0:D, (og % 4) * 1024:(og % 4 + 1) * 1024]
                        if og < 16:
                            v.tensor_copy(Tn[:, og * 1024:(og + 1) * 1024], src
                                          ).then_inc(s_tnv)
                        else:
                            v.tensor_copy(Xs[:, (og - 16) * 1024:(og - 15) * 1024],
                                          src).then_inc(s_tnv)

                @block.scalar
                def _(sc):
                    sc.wait_ge(s_stat, 2)
                    sc.activation(normb[:, :], n2c[:, :], AF.Sqrt)
                    sc.activation(xnormb[:, :], xn2c[:, :], AF.Sqrt
                                  ).then_inc(s_sqrt)
                    for og in range(18):
                        if out_eng[og] != "a":
                            continue
                        sc.wait_ge(s_tno, og + 1)
                        src = ps16[0:D, (og % 4) * 1024:(og % 4 + 1) * 1024]
                        sc.copy(Tn[:, og * 1024:(og + 1) * 1024], src
                                ).then_inc(s_tna)

            if NB == 0:
                # bisect stage "pre": preprocessing only
                @block.sync
                def _(sy):
                    sy.wait_ge(s_tnv, TOT_V)
                    sy.wait_ge(s_tna, TOT_A)
                    sy.dma_start(out=out_d[:, :], in_=epsnx[0:1, 0:1]
                                 ).then_inc(s_dma, 16)

            # ================= PHASE 2: main loop =================
            if NB > 0:
              with ExitStack() as p2:
                sb2 = lambda name, shape, dt: p2.enter_context(
                    nc.sbuf_tensor(name, shape, dt))
                S0 = sb2("S0", [128, N], F16)
                S1 = sb2("S1", [128, N], F16)
                wbuf = [sb2(f"wbuf{i}", [128, 4096], F16) for i in range(4)]
                TA = sb2("TA", [128, 8192], F16)
                TB = sb2("TB", [128, 4096], F16)
                TAm = sb2("TAm", [128, 5120], F16)
                TBm = sb2("TBm", [128, 2560], F16)
                psB = p2.enter_context(nc.psum_tensor("psB", [128, 4096], F32))
                Sb = [S0, S1]

                @block.tensor
                def _(pe):
                    pe.wait_ge(s_tnv, TOT_V)
                    pe.wait_ge(s_tna, TOT_A)
                    for n in range(NB):
                        for g in range(NG):
                            K = n * NG + g
                            if K >= 2:
                                prev = K - 2
                                if ext_is_act(prev % NG):
                                    pe.wait_ge(s_ea, act_cum[prev])
                                else:
                                    pe.wait_ge(s_ev, dve_cum[prev])
                            for c in range(4):
                                q = g * 4 + c
                                col = q * 512
                                ins = pe.matmul(
                                    psB[:, (q % 8) * 512:(q % 8 + 1) * 512],
                                    Xs[:, n * 128:(n + 1) * 128],
                                    Tn[:, col:col + 512],
                                )
                            # one inc per 2048-wide group: keeps the sem-inc
                            # rate low (cayman event-accel deadlock errata)
                            ins.then_inc(s_mm)

                def trees_and_b_thunks(v, n):
                    """row-min/max + temperature b for row block n (on DVE).

                    Chain thunks: consecutive entries depend on each other,
                    so they must be interleaved with an unrelated list."""
                    S = Sb[n % 2]
                    par = n % 2

                    def start():
                        v.wait_ge(s_ea, NGA * (n + 1))
                        v.wait_ge(s_ev, len(GD) * (n + 1))

                    tmin = _tree_thunks(v, S[:, 0:NGA * GRP], NGA * GRP, OP.min,
                                        mtree[:, :], TAm, TBm)
                    tmax = _tree_thunks(v, S[:, 0:N], N, OP.max,
                                        Mall[:, n:n + 1], TA, TB)
                    tmin[0] = (lambda f=tmin[0]: (start(), f()))
                    chain = [
                        lambda: v.tensor_tensor(mc1[:, :], minc[:, 0:1],
                                                minc[:, 1:2], op=OP.min),
                        lambda: v.tensor_tensor(mc2[:, :], mc1[:, :],
                                                minc[:, 2:3], op=OP.min),
                        lambda: v.tensor_tensor(mall[:, n:n + 1], mtree[:, :],
                                                mc2[:, :], op=OP.min),
                        lambda: v.tensor_add(mp[:, :], mall[:, n:n + 1],
                                             epsnx[:, n:n + 1]),
                        lambda: v.tensor_scalar_mul(dd[:, :], mp[:, :], -H),
                        lambda: v.reciprocal(ball[:, n:n + 1], dd[:, :]),
                        lambda: v.tensor_mul(Upar[par][:, 0:1],
                                             ball[:, n:n + 1], Mall[:, n:n + 1]),
                        lambda: v.tensor_mul(Upar[par][:, 1:2],
                                             ball[:, n:n + 1], mall[:, n:n + 1]
                                             ).then_inc(s_b),
                    ]
                    # tmin feeds chain[0]; keep them in one serial list
                    return tmin + chain, tmax

                def r_block_thunks(v, n):
                    """r_n = max(exp(bM), exp(bm)) / Z for row block n (DVE)."""
                    par = n % 2
                    return [
                        lambda: (v.wait_ge(s_z, n + 1),
                                 v.reduce_sum(Zall[:, n:n + 1], Zpar[par][:, :],
                                              axis=AX.X)),
                        lambda: v.reciprocal(zr[:, :], Zall[:, n:n + 1]),
                        lambda: v.tensor_tensor(wm[:, :], Epar[par][:, 0:1],
                                                Epar[par][:, 1:2], op=OP.max),
                        lambda: v.tensor_mul(r_all[:, n:n + 1], wm[:, :],
                                             zr[:, :]),
                    ]

                @block.vector
                def _(v):
                    for n in range(NB):
                        if n >= 1:
                            a, b = trees_and_b_thunks(v, n - 1)
                            if n >= 2:
                                b = b + r_block_thunks(v, n - 2)
                            _interleave(v, a, b)
                        for j, g in enumerate(GD):
                            v.wait_ge(s_mm, n * NG + g + 1)
                            v.tensor_scalar(
                                out=Sb[n % 2][:, g * GRP:(g + 1) * GRP],
                                in0=psB[:, (g % 2) * GRP:(g % 2) * GRP + GRP],
                                scalar1=0.0,
                                scalar2=None,
                                op0=OP.add,
                                op1=OP.min,
                                accum_out=minc[:, j:j + 1],
                            ).then_inc(s_ev)
                    a, b = trees_and_b_thunks(v, NB - 1)
                    tail = r_block_thunks(v, NB - 2) if NB >= 2 else []
                    _interleave(v, a, b + tail)
                    _interleave(v, r_block_thunks(v, NB - 1))
                    v.drain()
                    v.reduce_max(rmaxb[:, :], r_all[:, 0:NB], axis=AX.X
                                 ).then_inc(s_r)
                    if stage == "full":
                        # finale: negate log
                        v.wait_ge(s_fin, 2)
                        v.tensor_scalar_mul(outsb[:, :], lnb[:, :], -1.0
                                            ).then_inc(s_fin)   # -> 3

                def exp_block(sc, n):
                    sc.wait_ge(s_b, n + 1)
                    sc.wait_ge(s_ea, NGA * (n + 1))
                    sc.wait_ge(s_ev, len(GD) * (n + 1))
                    if n >= 2:   # certify WAW on wbuf/Zpar/Epar vs block n-2
                        sc.wait_ge(s_z, n - 1)
                    par = n % 2
                    for q in range(4):
                        sc.activation(
                            wbuf[q][:, :],
                            Sb[par][:, q * 4096:(q + 1) * 4096],
                            AF.Exp,
                            scale=ball[:, n:n + 1],
                            accum_out=Zpar[par][:, q:q + 1],
                        )
                    sc.activation(Epar[par][:, :], Upar[par][:, :], AF.Exp
                                  ).then_inc(s_z)

                @block.scalar
                def _(sc):
                    for n in range(NB):
                        if n >= 2:
                            # S[n%2] was read by exp(n-2); certify those reads
                            # (small exp of block n-2 incs s_z to n-1)
                            sc.wait_ge(s_z, n - 1)
                        for g in range(NGA):
                            sc.wait_ge(s_mm, n * NG + g + 1)
                            sc.copy(Sb[n % 2][:, g * GRP:(g + 1) * GRP],
                                    psB[:, (g % 2) * GRP:(g % 2) * GRP + GRP]
                                    ).then_inc(s_ea)
                        if n >= 1:
                            exp_block(sc, n - 1)
                    exp_block(sc, NB - 1)
                    if stage == "full":
                        # finale: ln
                        sc.wait_ge(s_cdma, 32)
                        sc.activation(lnb[:, :], gmb[:, :], AF.Ln
                                      ).then_inc(s_fin)

                @block.gpsimd
                def _(gp):
                    if stage != "full":
                        return
                    from concourse import library_config
                    gp.load_library(library_config.attn)
                    gp.wait_ge(s_r, 1)
                    gp.partition_all_reduce(prb[:, :], rmaxb[:, :], 128,
                                            bass_isa.ReduceOp.max
                                            ).then_inc(s_r)   # -> 2
                    gp.wait_ge(s_r, 2)
                    gp.dma_start(out=cc_in[:, :], in_=prb[0:1, 0:1]
                                 ).then_inc(s_cdma, 16)
                    gp.wait_ge(s_cdma, 16)
                    gp.collective_compute(
                        "AllReduce",
                        OP.max,
                        replica_groups=[list(range(P))],
                        ins=[cc_in[:, :]],
                        outs=[cc_out[:, :]],
                    ).then_inc(s_fin)   # -> 1
                    gp.wait_ge(s_fin, 1)
                    gp.dma_start(out=gmb[:, :], in_=cc_out[:, :]
                                 ).then_inc(s_cdma, 16)   # -> 32

                @block.sync
                def _(sy):
                    if stage == "full":
                        sy.wait_ge(s_fin, 3)
                        sy.dma_start(out=out_d[:, :], in_=outsb[:, :]
                                     ).then_inc(s_dma, 16)
                    else:
                        sy.wait_ge(s_r, 1)
                        sy.dma_start(out=out_d[:, :], in_=rmaxb[0:1, 0:1]
                                     ).then_inc(s_dma, 16)

    # populate .instr bytes for extended-ISA instructions (TTR,
    # partition_all_reduce) — raw Bass skips this pass otherwise and the
    # NEFF compiler fails with "ISA wrong length"
    from concourse.library_overlay import lower_extended_insts
    lower_extended_insts(nc)
    return nc


_NC = None


def _get_nc():
    global _NC
    if _NC is None:
        _NC = build()
    return _NC


LAST_RESULT = None


def kernel(input, target_features, **bench_kwargs):
    global LAST_RESULT
    from concourse.bass_utils import run_bass_kernel_spmd

    X = np.ascontiguousarray(
        np.asarray(input, dtype=np.float32).reshape(50, N)[10:50])
    T = np.ascontiguousarray(
        np.asarray(target_features, dtype=np.float32).reshape(50, N)[10:50])
    in_maps = []
    for r in range(P):
        in_maps.append({
            "xs": np.ascontiguousarray(X[:, r * SH:(r + 1) * SH]),
            "ts": np.ascontiguousarray(T[:, r * SH:(r + 1) * SH]),
            "t": T,
        })
    nc = _get_nc()
    res = run_bass_kernel_spmd(nc, in_maps, core_ids=list(range(P)),
                               **bench_kwargs)
    LAST_RESULT = res
    return np.asarray(res.results[0]["out"], dtype=np.float32).reshape(())


# revision 40
# speedup vs baseline: 1.1674x; 1.1674x over previous
"""Trainium2 Bass kernel for nn_ContextualLoss (8 NeuronCores, SPMD).

Math (derived from the reference):
  X = vec(input)[10:50] - mu,  T = vec(target)[10:50] - mu,  mu = colmean(target)
  S[i,j] = cos(x_i, t_j);  CX = softmax_j(a_i * S[i,j]),  a_i = -1/(h*(min_j S + eps))
  loss = -log(max_ij CX)
X's column normalization folds into the softmax temperature:
  logits = b_i * R[i,j],  R = Xc^T Tn  (Xc centered only, Tn column-normalized)
  b_i = -1/(h*(min_j R + eps*||x_i||))
Per row we need: min_j R (f32/fp16), max_j R (fp16), sum_j exp(b_i R) (f32 acc).
Final: per-core max of exp(b*M)/Z, AllReduce-max over 8 cores, -log.

Sharding: each core computes 2048 of the 16384 S-rows (x-columns).
"""

import numpy as np
from contextlib import ExitStack

import concourse.bass as bass
import concourse.mybir as mybir
from concourse import bass_isa

F32 = mybir.dt.float32
F16 = mybir.dt.float16
AF = mybir.ActivationFunctionType
OP = mybir.AluOpType
AX = mybir.AxisListType

D = 40          # contraction dim (rows 10:50)
N = 16384       # feature columns
P = 8           # cores
SH = N // P     # x-columns per core
NRB = SH // 128  # row blocks per core = 16
NG = 8          # 2048-wide column groups per row block
NGA = 5         # groups extracted by ScalarE (plain copy)
GD = [5, 6, 7]  # groups extracted by VectorE (TTR copy+min)
GRP = 2048      # group width
EPS = 1e-5
H = 0.2
BIGF = 3.0e38


def _tree_thunks(v, src, width, op, out_slice, TA, TB, stop=512):
    """Thunks for a pairwise-halving fp16 reduce on VectorE.

    Returned thunks must be interleaved with >=1 unrelated op between
    consecutive ones (DVE pipeline output hazard: an op may not read the
    immediately-preceding op's output without a drain)."""
    thunks = []
    cur, w = src, width
    bufs = [TA, TB]
    bi = 0
    while w > stop:
        h = w // 2
        dst = bufs[bi]
        thunks.append(lambda cur=cur, h=h, w=w, dst=dst:
                      v.tensor_tensor(dst[:, 0:h], cur[:, 0:h], cur[:, h:w],
                                      op=op))
        cur, w = bufs[bi], h
        bi ^= 1
    thunks.append(lambda cur=cur, w=w:
                  v.tensor_reduce(out_slice, cur[:, 0:w], axis=AX.X, op=op))
    return thunks


def _interleave(v, *lists):
    """Emit ops round-robin from the lists with a drain between rounds.

    The race model requires every same-engine RAW/WAW pair to be separated
    by a drain (or a sem-certified wait); ops within one round are mutually
    independent, so one drain per round suffices and overlaps round issue."""
    n = max(len(l) for l in lists)
    for i in range(n):
        for l in lists:
            if i < len(l):
                l[i]()
        if i < n - 1:
            v.drain()


def build():
    import os
    stage = os.environ.get("K_STAGE", "full")   # debug bisect hook
    if stage == "pre":
        NB = 0
    elif stage.startswith("main"):
        NB = int(stage[4:])
    else:
        NB = NRB

    nc = bass.Bass(num_devices=P)

    xs_d = nc.declare_dram_parameter("xs", [D, SH], F32, isOutput=False)
    ts_d = nc.declare_dram_parameter("ts", [D, SH], F32, isOutput=False)
    t_d = nc.declare_dram_parameter("t", [D, N], F32, isOutput=False)
    out_d = nc.declare_dram_parameter("out", [1, 1], F32, isOutput=True)

    eye40_d = nc.inline_tensor(np.eye(D, dtype=np.float32), "eye40c")
    eye128_d = nc.inline_tensor(np.eye(128, dtype=np.float16), "eye128c")
    cc_in = nc.dram_tensor("cc_in", [1, 1], F32)
    cc_out = nc.dram_tensor("cc_out", [1, 1], F32, addr_space="Shared")

    ctx = ExitStack()
    with ctx:
        sbuf = lambda name, shape, dt: ctx.enter_context(
            nc.sbuf_tensor(name, shape, dt))
        sem = lambda name: ctx.enter_context(nc.semaphore(name))

        # ---- persistent SBUF ----
        eye40 = sbuf("eye40", [D, D], F32)
        eye128 = sbuf("eye128", [128, 128], F16)
        Tn = sbuf("Tn", [D, N], F16)         # normalized target, matmul rhs
        Xs = sbuf("Xsb", [D, SH], F16)       # centered x shard, matmul lhsT
        zeros = sbuf("zeros", [128, GRP], F32)
        epsnx = sbuf("epsnx", [128, NRB], F32)
        Mall = sbuf("Mall", [128, NRB], F32)
        mall = sbuf("mall", [128, NRB], F32)
        ball = sbuf("ball", [128, NRB], F32)
        Zall = sbuf("Zall", [128, NRB], F32)
        r_all = sbuf("r_all", [128, NRB], F32)
        minc = sbuf("minc", [128, len(GD)], F32)
        Upar = [sbuf(f"U{i}", [128, 2], F32) for i in range(2)]
        Epar = [sbuf(f"E{i}", [128, 2], F32) for i in range(2)]
        Zpar = [sbuf(f"Zp{i}", [128, 4], F32) for i in range(2)]
        mtree = sbuf("mtree", [128, 1], F32)
        mc1 = sbuf("mc1", [128, 1], F32)
        mc2 = sbuf("mc2", [128, 1], F32)
        mp = sbuf("mp", [128, 1], F32)
        dd = sbuf("dd", [128, 1], F32)
        zr = sbuf("zr", [128, 1], F32)
        wm = sbuf("wm", [128, 1], F32)
        rmaxb = sbuf("rmaxb", [128, 1], F32)
        prb = sbuf("prb", [128, 1], F32)
        gmb = sbuf("gmb", [1, 1], F32)
        lnb = sbuf("lnb", [1, 1], F32)
        outsb = sbuf("outsb", [1, 1], F32)

        s_dma = sem("s_dma")
        s_e40 = sem("s_e40")
        s_e128 = sem("s_e128")
        s_xs = sem("s_xs")
        s_ts = sem("s_ts")
        s_tc = [sem("s_tc0"), sem("s_tc1")]
        s_tp = sem("s_tp")
        s_tpc = sem("s_tpc")
        s_stat = sem("s_stat")
        s_sqrt = sem("s_sqrt")
        s_tno = sem("s_tno")
        s_tnv = sem("s_tnv")
        s_tna = sem("s_tna")
        s_mm = sem("s_mm")
        s_ev = sem("s_ev")
        s_ea = sem("s_ea")
        s_b = sem("s_b")
        s_z = sem("s_z")
        s_r = sem("s_r")
        s_fin = sem("s_fin")
        s_cdma = sem("s_cdma")

        # outward-copy engine assignment: og 0..15 = Tn groups, 16..17 = Xs
        out_eng = ["v" if (og % 8) < 5 else "a" for og in range(16)] + ["v", "v"]
        # emission order: X groups first so the main loop can start early
        emit_order = [16, 17] + list(range(16))
        # cumulative per-engine outward-copy counts in EMISSION order, by og
        ocv_e, oca_e = {}, {}
        cv = ca = 0
        for og in emit_order:
            if out_eng[og] == "v":
                cv += 1
            else:
                ca += 1
            ocv_e[og] = cv
            oca_e[og] = ca
        TOT_V, TOT_A = cv, ca

        # main-loop extraction engine per group index (within rb)
        def ext_is_act(g):
            return g < NGA

        # cumulative extraction counts after global group K
        act_cum, dve_cum = [], []
        a = v = 0
        for K in range(NRB * NG):
            if ext_is_act(K % NG):
                a += 1
            else:
                v += 1
            act_cum.append(a)
            dve_cum.append(v)

        with nc.Block() as block:
            # ================= PHASE 1: load + preprocess =================
            with ExitStack() as p1:
                sb1 = lambda name, shape, dt: p1.enter_context(
                    nc.sbuf_tensor(name, shape, dt))
                Traw0 = sb1("Traw0", [D, 4096], F32)
                Traw1 = sb1("Traw1", [D, 4096], F32)
                Xraw = sb1("Xraw", [D, SH], F32)
                Tsraw = sb1("Tsraw", [D, SH], F32)
                Tt = sb1("Tt", [128, 5120], F32)
                Tc = sb1("Tc", [128, 5120], F32)
                sqb = sb1("sqb", [128, 5120], F32)
                Tnt = sb1("Tnt", [128, 5120], F16)
                Xt = sb1("Xt", [128, 640], F32)
                Xct = sb1("Xct", [128, 640], F32)
                xsqb = sb1("xsqb", [128, 640], F32)
                Xct16 = sb1("Xct16", [128, 640], F16)
                Tst = sb1("Tst", [128, 640], F32)
                muS = sb1("muS", [128, 128], F32)
                mub = sb1("mub", [128, 128], F32)
                n2S = sb1("n2S", [128, 128], F32)
                n2c = sb1("n2c", [128, 128], F32)
                normb = sb1("normb", [128, 128], F32)
                y0b = sb1("y0b", [128, 128], F32)
                t1b = sb1("t1b", [128, 128], F32)
                t2b = sb1("t2b", [128, 128], F32)
                t3b = sb1("t3b", [128, 128], F32)
                rTb = sb1("rTb", [128, 128], F32)
                xmuS = sb1("xmuS", [128, NRB], F32)
                xmub = sb1("xmub", [128, NRB], F32)
                xn2S = sb1("xn2S", [128, NRB], F32)
                xn2c = sb1("xn2c", [128, NRB], F32)
                xnormb = sb1("xnormb", [128, NRB], F32)
                psT = p1.enter_context(nc.psum_tensor("psT", [128, 2048], F32))
                ps16 = p1.enter_context(nc.psum_tensor("ps16", [128, 4096], F16))
                Traws = [Traw0, Traw1]
                Tt3 = Tt[:, :].rearrange("p (t d) -> p t d", d=D)
                Tc3 = Tc[:, :].rearrange("p (t d) -> p t d", d=D)
                sq3 = sqb[:, :].rearrange("p (t d) -> p t d", d=D)
                Tnt3 = Tnt[:, :].rearrange("p (t d) -> p t d", d=D)
                Xt3 = Xt[:, :].rearrange("p (t d) -> p t d", d=D)
                Xct3 = Xct[:, :].rearrange("p (t d) -> p t d", d=D)
                xsq3 = xsqb[:, :].rearrange("p (t d) -> p t d", d=D)
                Tst3 = Tst[:, :].rearrange("p (t d) -> p t d", d=D)

                @block.sync
                def _(sy):
                    sy.dma_start(out=eye40[:, :], in_=eye40_d[:, :]).then_inc(s_e40, 16)
                    sy.dma_start(out=eye128[:, :], in_=eye128_d[:, :]).then_inc(s_e128, 16)
                    sy.dma_start(out=Xraw[:, :], in_=xs_d[:, :]).then_inc(s_xs, 16)
                    sy.dma_start(out=Tsraw[:, :], in_=ts_d[:, :]).then_inc(s_ts, 16)
                    for ci in range(4):
                        if ci >= 2:
                            # buffer reuse: transposes of chunk ci-2 done
                            sy.wait_ge(s_tp, 4 + 4 * (ci - 1))
                        sy.dma_start(
                            out=Traws[ci % 2][:, :],
                            in_=t_d[:, ci * 4096:(ci + 1) * 4096],
                        ).then_inc(s_tc[ci % 2], 16)

                @block.tensor
                def _(pe):
                    # inward transposes: [40,128] f32 tiles -> [128,40] in psT
                    # groups of 8 tiles; psT bank = grp % 4
                    def inw(grp, src_ap_fn):
                        if grp >= 4:
                            pe.wait_ge(s_tpc, grp - 3)
                        for k in range(8):
                            ins = pe.transpose(
                                psT[:, (grp % 4) * 512 + k * D:
                                    (grp % 4) * 512 + (k + 1) * D],
                                src_ap_fn(k),
                                eye40[:, :],
                            )
                        ins.then_inc(s_tp)

                    pe.wait_ge(s_e40, 16)
                    pe.wait_ge(s_e128, 16)
                    pe.wait_ge(s_xs, 16)
                    for g in range(2):
                        inw(g, lambda k, g=g: Xraw[:, (g * 8 + k) * 128:
                                                   (g * 8 + k + 1) * 128])
                    pe.wait_ge(s_ts, 16)
                    for g in range(2):
                        inw(2 + g, lambda k, g=g: Tsraw[:, (g * 8 + k) * 128:
                                                        (g * 8 + k + 1) * 128])
                    for ci in range(4):
                        pe.wait_ge(s_tc[ci % 2], 16 * (ci // 2 + 1))
                        for j in range(4):
                            inw(4 + ci * 4 + j,
                                lambda k, j=j, ci=ci: Traws[ci % 2][
                                    :, (j * 8 + k) * 128:(j * 8 + k + 1) * 128])

                    # outward transposes: [128,40] f16 tiles -> [40,128] in ps16
                    def outw(slot, og, src3, base_tile):
                        if slot >= 4:
                            prev = emit_order[slot - 4]
                            eng = out_eng[prev]
                            pe.wait_ge(s_tnv if eng == "v" else s_tna,
                                       ocv_e[prev] if eng == "v" else oca_e[prev])
                        for k in range(8):
                            t = base_tile + k
                            ins = pe.transpose(
                                ps16[0:D, (slot % 4) * 1024 + k * 128:
                                     (slot % 4) * 1024 + (k + 1) * 128],
                                src3[:, t * D:(t + 1) * D],
                                eye128[:, :],
                            )
                        ins.then_inc(s_tno)

                    pe.wait_ge(s_stat, 3)      # Xct16 ready
                    for slot, og in enumerate(emit_order):
                        if og == 16:
                            pass
                        elif og == 17:
                            pass
                        elif og == 0:
                            pe.wait_ge(s_stat, 4)      # Tnt ready
                        if og >= 16:
                            outw(slot, og, Xct16[:, :], (og - 16) * 8)
                        else:
                            outw(slot, og, Tnt[:, :], og * 8)

                @block.vector
                def _(v):
                    v.memset(zeros[:, :], 0.0)
                    v.drain()
                    # inward copies psT bank -> transposed layouts
                    for g in range(20):
                        v.wait_ge(s_tp, g + 1)
                        if g < 2:
                            dest = Xt[:, g * 320:(g + 1) * 320]
                        elif g < 4:
                            dest = Tst[:, (g - 2) * 320:(g - 1) * 320]
                        else:
                            dest = Tt[:, (g - 4) * 320:(g - 3) * 320]
                        v.tensor_copy(dest, psT[:, (g % 4) * 512:(g % 4) * 512 + 320]
                                      ).then_inc(s_tpc)
                    # stats, pipelined per 32-tile slab as T chunks land
                    xmu_bc = xmub[:, :].unsqueeze(2).broadcast_to((128, NRB, D))

                    def slab_ops(ci):
                        s32 = slice(ci * 32, (ci + 1) * 32)
                        t3 = Tt3[:, s32, :]
                        c3 = Tc3[:, s32, :]
                        q3 = sq3[:, s32, :]
                        mbc = mub[:, s32].unsqueeze(2).broadcast_to((128, 32, D))
                        return [
                            lambda: (v.wait_ge(s_tpc, 4 + 4 * (ci + 1)),
                                     v.reduce_sum(muS[:, s32], t3, axis=AX.X)),
                            lambda: v.tensor_scalar_mul(mub[:, s32],
                                                        muS[:, s32], 1.0 / D),
                            lambda: v.tensor_sub(c3, t3, mbc),
                            lambda: v.tensor_mul(q3, c3, c3),
                            lambda: v.reduce_sum(n2S[:, s32], q3, axis=AX.X),
                        ]

                    xchain = [
                        lambda: (v.wait_ge(s_tpc, 4),
                                 v.reduce_sum(xmuS[:, :], Tst3, axis=AX.X)),
                        lambda: v.tensor_scalar_mul(xmub[:, :], xmuS[:, :],
                                                    1.0 / D),
                        lambda: v.tensor_sub(Xct3, Xt3, xmu_bc),
                        lambda: v.tensor_mul(xsq3, Xct3, Xct3),
                        lambda: v.reduce_sum(xn2S[:, :], xsq3, axis=AX.X),
                    ]
                    _interleave(v, xchain, slab_ops(0))
                    _interleave(v, slab_ops(1))
                    _interleave(v, slab_ops(2))
                    _interleave(v, slab_ops(3))
                    v.drain()
                    clip = [
                        lambda: v.tensor_scalar_max(n2c[:, :], n2S[:, :], 1e-16
                                                    ).then_inc(s_stat),  # -> 1
                        lambda: v.tensor_scalar_max(xn2c[:, :], xn2S[:, :],
                                                    1e-16).then_inc(s_stat),  # 2
                    ]
                    _interleave(v, clip)
                    # rsqrt refine (needs ACT sqrt of n2c -> normb)
                    v.wait_ge(s_sqrt, 1)
                    r_bc = rTb[:, :].unsqueeze(2).broadcast_to((128, 128, D))
                    nrchain = [
                        lambda: v.reciprocal(y0b[:, :], normb[:, :]),
                        lambda: v.tensor_mul(t1b[:, :], y0b[:, :], y0b[:, :]),
                        lambda: v.tensor_mul(t2b[:, :], t1b[:, :], n2c[:, :]),
                        lambda: v.tensor_scalar(t3b[:, :], t2b[:, :], -0.5, 1.5,
                                                op0=OP.mult, op1=OP.add),
                        lambda: v.tensor_mul(rTb[:, :], y0b[:, :], t3b[:, :]),
                        lambda: v.tensor_mul(Tnt3, Tc3, r_bc
                                             ).then_inc(s_stat),   # -> 4
                    ]
                    othchain = [
                        lambda: v.tensor_scalar_mul(epsnx[:, :], xnormb[:, :],
                                                    EPS),
                        lambda: v.tensor_copy(Xct16[:, :], Xct[:, :]
                                              ).then_inc(s_stat),  # -> 3
                    ]
                    _interleave(v, nrchain, othchain)
                    # outward copies (vector share)
                    for slot, og in enumerate(emit_order):
                        if out_eng[og] != "v":
                            continue
                        v.wait_ge(s_tno, slot + 1)
                        src = ps16[0:D, (slot % 4) * 1024:(slot % 4 + 1) * 1024]
                        if og < 16:
                            v.tensor_copy(Tn[:, og * 1024:(og + 1) * 1024], src
                                          ).then_inc(s_tnv)
                        else:
                            v.tensor_copy(Xs[:, (og - 16) * 1024:(og - 15) * 1024],
                                          src).then_inc(s_tnv)

                @block.scalar
                def _(sc):
                    sc.wait_ge(s_stat, 2)
                    sc.activation(normb[:, :], n2c[:, :], AF.Sqrt)
                    sc.activation(xnormb[:, :], xn2c[:, :], AF.Sqrt
                                  ).then_inc(s_sqrt)
                    for slot, og in enumerate(emit_order):
                        if out_eng[og] != "a":
                            continue
                        sc.wait_ge(s_tno, slot + 1)
                        src = ps16[0:D, (slot % 4) * 1024:(slot % 4 + 1) * 1024]
                        sc.copy(Tn[:, og * 1024:(og + 1) * 1024], src
                                ).then_inc(s_tna)

            if NB == 0:
                # bisect stage "pre": preprocessing only
                @block.sync
                def _(sy):
                    sy.wait_ge(s_tnv, TOT_V)
                    sy.wait_ge(s_tna, TOT_A)
                    sy.dma_start(out=out_d[:, :], in_=epsnx[0:1, 0:1]
                                 ).then_inc(s_dma, 16)

            # ================= PHASE 2: main loop =================
            if NB > 0:
              with ExitStack() as p2:
                sb2 = lambda name, shape, dt: p2.enter_context(
                    nc.sbuf_tensor(name, shape, dt))
                S0 = sb2("S0", [128, N], F16)
                S1 = sb2("S1", [128, N], F16)
                wbuf = sb2("wbuf", [128, N], F16)
                TA = sb2("TA", [128, 8192], F16)
                TB = sb2("TB", [128, 4096], F16)
                TAm = sb2("TAm", [128, 5120], F16)
                TBm = sb2("TBm", [128, 2560], F16)
                psB = p2.enter_context(nc.psum_tensor("psB", [128, 4096], F32))
                Sb = [S0, S1]

                @block.tensor
                def _(pe):
                    pe.wait_ge(s_tnv, ocv_e[1])   # Xs + Tn cols [0, 4096)
                    if oca_e[1]:
                        pe.wait_ge(s_tna, oca_e[1])
                    for n in range(NB):
                        for g in range(NG):
                            K = n * NG + g
                            if K == 1:
                                # banks 4-7 were phase-1 ps16; also gates the
                                # rest of Tn
                                pe.wait_ge(s_tnv, TOT_V)
                                pe.wait_ge(s_tna, TOT_A)
                            if K >= 2:
                                prev = K - 2
                                if ext_is_act(prev % NG):
                                    pe.wait_ge(s_ea, act_cum[prev])
                                else:
                                    pe.wait_ge(s_ev, dve_cum[prev])
                            for c in range(4):
                                q = g * 4 + c
                                col = q * 512
                                ins = pe.matmul(
                                    psB[:, (q % 8) * 512:(q % 8 + 1) * 512],
                                    Xs[:, n * 128:(n + 1) * 128],
                                    Tn[:, col:col + 512],
                                )
                            # one inc per 2048-wide group: keeps the sem-inc
                            # rate low (cayman event-accel deadlock errata)
                            ins.then_inc(s_mm)

                def trees_and_b_thunks(v, n):
                    """row-min/max + temperature b for row block n (on DVE).

                    Chain thunks: consecutive entries depend on each other,
                    so they must be interleaved with an unrelated list."""
                    S = Sb[n % 2]
                    par = n % 2

                    def start():
                        v.wait_ge(s_ea, NGA * (n + 1))
                        v.wait_ge(s_ev, len(GD) * (n + 1))

                    tmin = _tree_thunks(v, S[:, 0:NGA * GRP], NGA * GRP, OP.min,
                                        mtree[:, :], TAm, TBm)
                    tmax = _tree_thunks(v, S[:, 0:N], N, OP.max,
                                        Mall[:, n:n + 1], TA, TB)
                    tmin[0] = (lambda f=tmin[0]: (start(), f()))
                    chain = [
                        lambda: v.tensor_tensor(mc1[:, :], minc[:, 0:1],
                                                minc[:, 1:2], op=OP.min),
                        lambda: v.tensor_tensor(mc2[:, :], mc1[:, :],
                                                minc[:, 2:3], op=OP.min),
                        lambda: v.tensor_tensor(mall[:, n:n + 1], mtree[:, :],
                                                mc2[:, :], op=OP.min),
                        lambda: v.tensor_add(mp[:, :], mall[:, n:n + 1],
                                             epsnx[:, n:n + 1]),
                        lambda: v.tensor_scalar_mul(dd[:, :], mp[:, :], -H),
                        lambda: v.reciprocal(ball[:, n:n + 1], dd[:, :]),
                        lambda: v.tensor_mul(Upar[par][:, 0:1],
                                             ball[:, n:n + 1], Mall[:, n:n + 1]),
                        lambda: v.tensor_mul(Upar[par][:, 1:2],
                                             ball[:, n:n + 1], mall[:, n:n + 1]
                                             ).then_inc(s_b),
                    ]
                    # tmin feeds chain[0]; keep them in one serial list
                    return tmin + chain, tmax

                def r_block_thunks(v, n):
                    """r_n = max(exp(bM), exp(bm)) / Z for row block n (DVE)."""
                    par = n % 2
                    return [
                        lambda: (v.wait_ge(s_z, n + 1),
                                 v.reciprocal(zr[:, :], Zall[:, n:n + 1])),
                        lambda: v.tensor_tensor(wm[:, :], Epar[par][:, 0:1],
                                                Epar[par][:, 1:2], op=OP.max),
                        lambda: v.tensor_mul(r_all[:, n:n + 1], wm[:, :],
                                             zr[:, :]),
                    ]

                @block.vector
                def _(v):
                    for n in range(NB):
                        if n >= 1:
                            a, b = trees_and_b_thunks(v, n - 1)
                            if n >= 2:
                                b = b + r_block_thunks(v, n - 2)
                            _interleave(v, a, b)
                        for j, g in enumerate(GD):
                            v.wait_ge(s_mm, n * NG + g + 1)
                            v.tensor_scalar(
                                out=Sb[n % 2][:, g * GRP:(g + 1) * GRP],
                                in0=psB[:, (g % 2) * GRP:(g % 2) * GRP + GRP],
                                scalar1=0.0,
                                scalar2=None,
                                op0=OP.add,
                                op1=OP.min,
                                accum_out=minc[:, j:j + 1],
                            ).then_inc(s_ev)
                    a, b = trees_and_b_thunks(v, NB - 1)
                    tail = r_block_thunks(v, NB - 2) if NB >= 2 else []
                    _interleave(v, a, b + tail)
                    _interleave(v, r_block_thunks(v, NB - 1))
                    v.drain()
                    v.reduce_max(rmaxb[:, :], r_all[:, 0:NB], axis=AX.X
                                 ).then_inc(s_r)
                    if stage == "full":
                        # finale: negate log
                        v.wait_ge(s_fin, 2)
                        v.tensor_scalar_mul(outsb[:, :], lnb[:, :], -1.0
                                            ).then_inc(s_fin)   # -> 3

                def exp_block(sc, n):
                    sc.wait_ge(s_b, n + 1)
                    sc.wait_ge(s_ea, NGA * (n + 1))
                    sc.wait_ge(s_ev, len(GD) * (n + 1))
                    if n >= 1:   # certify WAW on wbuf/Epar vs prior blocks
                        sc.wait_ge(s_z, n)
                    par = n % 2
                    sc.activation(
                        wbuf[:, :],
                        Sb[par][:, :],
                        AF.Exp,
                        scale=ball[:, n:n + 1],
                        accum_out=Zall[:, n:n + 1],
                    )
                    sc.activation(Epar[par][:, :], Upar[par][:, :], AF.Exp
                                  ).then_inc(s_z)

                @block.scalar
                def _(sc):
                    for n in range(NB):
                        if n >= 2:
                            # S[n%2] was read by exp(n-2); certify those reads
                            # (small exp of block n-2 incs s_z to n-1)
                            sc.wait_ge(s_z, n - 1)
                        for g in range(NGA):
                            sc.wait_ge(s_mm, n * NG + g + 1)
                            sc.copy(Sb[n % 2][:, g * GRP:(g + 1) * GRP],
                                    psB[:, (g % 2) * GRP:(g % 2) * GRP + GRP]
                                    ).then_inc(s_ea)
                        if n >= 1:
                            exp_block(sc, n - 1)
                    exp_block(sc, NB - 1)
                    if stage == "full":
                        # finale: ln
                        sc.wait_ge(s_cdma, 32)
                        sc.activation(lnb[:, :], gmb[:, :], AF.Ln
                                      ).then_inc(s_fin)

                @block.gpsimd
                def _(gp):
                    if stage != "full":
                        return
                    from concourse import library_config
                    gp.load_library(library_config.attn)
                    gp.wait_ge(s_r, 1)
                    gp.partition_all_reduce(prb[:, :], rmaxb[:, :], 128,
                                            bass_isa.ReduceOp.max
                                            ).then_inc(s_r)   # -> 2
                    gp.wait_ge(s_r, 2)
                    gp.dma_start(out=cc_in[:, :], in_=prb[0:1, 0:1]
                                 ).then_inc(s_cdma, 16)
                    gp.wait_ge(s_cdma, 16)
                    gp.collective_compute(
                        "AllReduce",
                        OP.max,
                        replica_groups=[list(range(P))],
                        ins=[cc_in[:, :]],
                        outs=[cc_out[:, :]],
                    ).then_inc(s_fin)   # -> 1
                    gp.wait_ge(s_fin, 1)
                    gp.dma_start(out=gmb[:, :], in_=cc_out[:, :]
                                 ).then_inc(s_cdma, 16)   # -> 32

                @block.sync
                def _(sy):
                    if stage == "full":
                        sy.wait_ge(s_fin, 3)
                        sy.dma_start(out=out_d[:, :], in_=outsb[:, :]
                                     ).then_inc(s_dma, 16)
                    else:
                        sy.wait_ge(s_r, 1)
                        sy.dma_start(out=out_d[:, :], in_=rmaxb[0:1, 0:1]
                                     ).then_inc(s_dma, 16)

    # populate .instr bytes for extended-ISA instructions (TTR,
    # partition_all_reduce) — raw Bass skips this pass otherwise and the
    # NEFF compiler fails with "ISA wrong length"
    from concourse.library_overlay import lower_extended_insts
    lower_extended_insts(nc)
    return nc


_NC = None


def _get_nc():
    global _NC
    if _NC is None:
        _NC = build()
    return _NC


LAST_RESULT = None


def kernel(input, target_features, **bench_kwargs):
    global LAST_RESULT
    from concourse.bass_utils import run_bass_kernel_spmd

    X = np.ascontiguousarray(
        np.asarray(input, dtype=np.float32).reshape(50, N)[10:50])
    T = np.ascontiguousarray(
        np.asarray(target_features, dtype=np.float32).reshape(50, N)[10:50])
    in_maps = []
    for r in range(P):
        in_maps.append({
            "xs": np.ascontiguousarray(X[:, r * SH:(r + 1) * SH]),
            "ts": np.ascontiguousarray(T[:, r * SH:(r + 1) * SH]),
            "t": T,
        })
    nc = _get_nc()
    res = run_bass_kernel_spmd(nc, in_maps, core_ids=list(range(P)),
                               **bench_kwargs)
    LAST_RESULT = res
    return np.asarray(res.results[0]["out"], dtype=np.float32).reshape(())


# revision 42
# speedup vs baseline: 1.2539x; 1.0741x over previous
"""Trainium2 Bass kernel for nn_ContextualLoss (8 NeuronCores, SPMD).

Math (derived from the reference):
  X = vec(input)[10:50] - mu,  T = vec(target)[10:50] - mu,  mu = colmean(target)
  S[i,j] = cos(x_i, t_j);  CX = softmax_j(a_i * S[i,j]),  a_i = -1/(h*(min_j S + eps))
  loss = -log(max_ij CX)
X's column normalization folds into the softmax temperature:
  logits = b_i * R[i,j],  R = Xc^T Tn  (Xc centered only, Tn column-normalized)
  b_i = -1/(h*(min_j R + eps*||x_i||))
Per row we need: min_j R (f32/fp16), max_j R (fp16), sum_j exp(b_i R) (f32 acc).
Final: per-core max of exp(b*M)/Z, AllReduce-max over 8 cores, -log.

Sharding: each core computes 2048 of the 16384 S-rows (x-columns).
"""

import numpy as np
from contextlib import ExitStack

import concourse.bass as bass
import concourse.mybir as mybir
from concourse import bass_isa

F32 = mybir.dt.float32
F16 = mybir.dt.float16
AF = mybir.ActivationFunctionType
OP = mybir.AluOpType
AX = mybir.AxisListType

D = 40          # contraction dim (rows 10:50)
N = 16384       # feature columns
P = 8           # cores
SH = N // P     # x-columns per core
NRB = SH // 128  # row blocks per core = 16
NG = 8          # 2048-wide column groups per row block
NGA = 5         # groups extracted by ScalarE (plain copy)
GD = [5, 6, 7]  # groups extracted by VectorE (TTR copy+min)
GRP = 2048      # group width
EPS = 1e-5
H = 0.2
BIGF = 3.0e38


def _tree_thunks(v, src, width, op, out_slice, TA, TB, stop=512):
    """Thunks for a pairwise-halving fp16 reduce on VectorE.

    Returned thunks must be interleaved with >=1 unrelated op between
    consecutive ones (DVE pipeline output hazard: an op may not read the
    immediately-preceding op's output without a drain)."""
    thunks = []
    cur, w = src, width
    bufs = [TA, TB]
    bi = 0
    while w > stop:
        h = w // 2
        dst = bufs[bi]
        thunks.append(lambda cur=cur, h=h, w=w, dst=dst:
                      v.tensor_tensor(dst[:, 0:h], cur[:, 0:h], cur[:, h:w],
                                      op=op))
        cur, w = bufs[bi], h
        bi ^= 1
    thunks.append(lambda cur=cur, w=w:
                  v.tensor_reduce(out_slice, cur[:, 0:w], axis=AX.X, op=op))
    return thunks


def _interleave(v, *lists):
    """Emit ops round-robin from the lists with a drain between rounds.

    The race model requires every same-engine RAW/WAW pair to be separated
    by a drain (or a sem-certified wait); ops within one round are mutually
    independent, so one drain per round suffices and overlaps round issue."""
    n = max(len(l) for l in lists)
    for i in range(n):
        for l in lists:
            if i < len(l):
                l[i]()
        if i < n - 1:
            v.drain()


def build():
    import os
    stage = os.environ.get("K_STAGE", "full")   # debug bisect hook
    if stage == "pre":
        NB = 0
    elif stage.startswith("main"):
        NB = int(stage[4:])
    else:
        NB = NRB

    nc = bass.Bass(num_devices=P)

    xs_d = nc.declare_dram_parameter("xs", [D, SH], F32, isOutput=False)
    ts_d = nc.declare_dram_parameter("ts", [D, SH], F32, isOutput=False)
    t_d = nc.declare_dram_parameter("t", [D, N], F32, isOutput=False)
    out_d = nc.declare_dram_parameter("out", [1, 1], F32, isOutput=True)

    eye40_d = nc.inline_tensor(np.eye(D, dtype=np.float32), "eye40c")
    eye128_d = nc.inline_tensor(np.eye(128, dtype=np.float16), "eye128c")
    scr_d = nc.dram_tensor("scrmax", [128, 1], F32)

    ctx = ExitStack()
    with ctx:
        sbuf = lambda name, shape, dt: ctx.enter_context(
            nc.sbuf_tensor(name, shape, dt))
        sem = lambda name: ctx.enter_context(nc.semaphore(name))

        # ---- persistent SBUF ----
        eye40 = sbuf("eye40", [D, D], F32)
        eye128 = sbuf("eye128", [128, 128], F16)
        Tn = sbuf("Tn", [D, N], F16)         # normalized target, matmul rhs
        Xs = sbuf("Xsb", [D, SH], F16)       # centered x shard, matmul lhsT
        zeros = sbuf("zeros", [128, GRP], F32)
        epsnx = sbuf("epsnx", [128, NRB], F32)
        Mall = sbuf("Mall", [128, NRB], F32)
        mall = sbuf("mall", [128, NRB], F32)
        ball = sbuf("ball", [128, NRB], F32)
        Zall = sbuf("Zall", [128, NRB], F32)
        r_all = sbuf("r_all", [128, NRB], F32)
        minc = sbuf("minc", [128, len(GD)], F32)
        Upar = [sbuf(f"U{i}", [128, 2], F32) for i in range(2)]
        Epar = [sbuf(f"E{i}", [128, 2], F32) for i in range(2)]
        Zpar = [sbuf(f"Zp{i}", [128, 4], F32) for i in range(2)]
        mtree = sbuf("mtree", [128, 1], F32)
        mc1 = sbuf("mc1", [128, 1], F32)
        mc2 = sbuf("mc2", [128, 1], F32)
        mp = sbuf("mp", [128, 1], F32)
        dd = sbuf("dd", [128, 1], F32)
        zr = sbuf("zr", [128, 1], F32)
        wm = sbuf("wm", [128, 1], F32)
        rmaxb = sbuf("rmaxb", [128, 1], F32)
        rrow = sbuf("rrow", [1, 128], F32)
        gmx = sbuf("gmx", [1, 1], F32)
        prb = sbuf("prb", [128, 1], F32)
        gmb = sbuf("gmb", [1, 1], F32)
        lnb = sbuf("lnb", [1, 1], F32)
        outsb = sbuf("outsb", [1, 1], F32)

        s_dma = sem("s_dma")
        s_e40 = sem("s_e40")
        s_e128 = sem("s_e128")
        s_xs = sem("s_xs")
        s_ts = sem("s_ts")
        s_tc = [sem("s_tc0"), sem("s_tc1")]
        s_tp = sem("s_tp")
        s_tpc = sem("s_tpc")
        s_stat = sem("s_stat")
        s_sqrt = sem("s_sqrt")
        s_tno = sem("s_tno")
        s_tnv = sem("s_tnv")
        s_tna = sem("s_tna")
        s_mm = sem("s_mm")
        s_ev = sem("s_ev")
        s_ea = sem("s_ea")
        s_b = sem("s_b")
        s_z = sem("s_z")
        s_r = sem("s_r")
        s_fin = sem("s_fin")
        s_cdma = sem("s_cdma")

        # outward-copy engine assignment: og 0..15 = Tn groups, 16..17 = Xs
        out_eng = ["v" if (og % 8) < 5 else "a" for og in range(16)] + ["v", "v"]
        # emission order: X groups first so the main loop can start early
        emit_order = [16, 17] + list(range(16))
        # cumulative per-engine outward-copy counts in EMISSION order, by og
        ocv_e, oca_e = {}, {}
        cv = ca = 0
        for og in emit_order:
            if out_eng[og] == "v":
                cv += 1
            else:
                ca += 1
            ocv_e[og] = cv
            oca_e[og] = ca
        TOT_V, TOT_A = cv, ca

        # main-loop extraction engine per group index (within rb)
        def ext_is_act(g):
            return g < NGA

        # cumulative extraction counts after global group K
        act_cum, dve_cum = [], []
        a = v = 0
        for K in range(NRB * NG):
            if ext_is_act(K % NG):
                a += 1
            else:
                v += 1
            act_cum.append(a)
            dve_cum.append(v)

        with nc.Block() as block:
            # ================= PHASE 1: load + preprocess =================
            with ExitStack() as p1:
                sb1 = lambda name, shape, dt: p1.enter_context(
                    nc.sbuf_tensor(name, shape, dt))
                Traw0 = sb1("Traw0", [D, 4096], F32)
                Traw1 = sb1("Traw1", [D, 4096], F32)
                Xraw = sb1("Xraw", [D, SH], F32)
                Tsraw = sb1("Tsraw", [D, SH], F32)
                Tt = sb1("Tt", [128, 5120], F32)
                Tc = sb1("Tc", [128, 5120], F32)
                sqb = sb1("sqb", [128, 5120], F32)
                Tnt = sb1("Tnt", [128, 5120], F16)
                Xt = sb1("Xt", [128, 640], F32)
                Xct = sb1("Xct", [128, 640], F32)
                xsqb = sb1("xsqb", [128, 640], F32)
                Xct16 = sb1("Xct16", [128, 640], F16)
                Tst = sb1("Tst", [128, 640], F32)
                muS = sb1("muS", [128, 128], F32)
                mub = sb1("mub", [128, 128], F32)
                n2S = sb1("n2S", [128, 128], F32)
                n2c = sb1("n2c", [128, 128], F32)
                normb = sb1("normb", [128, 128], F32)
                y0b = sb1("y0b", [128, 128], F32)
                t1b = sb1("t1b", [128, 128], F32)
                t2b = sb1("t2b", [128, 128], F32)
                t3b = sb1("t3b", [128, 128], F32)
                rTb = sb1("rTb", [128, 128], F32)
                xmuS = sb1("xmuS", [128, NRB], F32)
                xmub = sb1("xmub", [128, NRB], F32)
                xn2S = sb1("xn2S", [128, NRB], F32)
                xn2c = sb1("xn2c", [128, NRB], F32)
                xnormb = sb1("xnormb", [128, NRB], F32)
                psT = p1.enter_context(nc.psum_tensor("psT", [128, 2048], F32))
                ps16 = p1.enter_context(nc.psum_tensor("ps16", [128, 4096], F16))
                Traws = [Traw0, Traw1]
                Tt3 = Tt[:, :].rearrange("p (t d) -> p t d", d=D)
                Tc3 = Tc[:, :].rearrange("p (t d) -> p t d", d=D)
                sq3 = sqb[:, :].rearrange("p (t d) -> p t d", d=D)
                Tnt3 = Tnt[:, :].rearrange("p (t d) -> p t d", d=D)
                Xt3 = Xt[:, :].rearrange("p (t d) -> p t d", d=D)
                Xct3 = Xct[:, :].rearrange("p (t d) -> p t d", d=D)
                xsq3 = xsqb[:, :].rearrange("p (t d) -> p t d", d=D)
                Tst3 = Tst[:, :].rearrange("p (t d) -> p t d", d=D)

                @block.sync
                def _(sy):
                    sy.dma_start(out=eye40[:, :], in_=eye40_d[:, :]).then_inc(s_e40, 16)
                    sy.dma_start(out=eye128[:, :], in_=eye128_d[:, :]).then_inc(s_e128, 16)
                    sy.dma_start(out=Xraw[:, :], in_=xs_d[:, :]).then_inc(s_xs, 16)
                    sy.dma_start(out=Tsraw[:, :], in_=ts_d[:, :]).then_inc(s_ts, 16)
                    for ci in range(4):
                        if ci >= 2:
                            # buffer reuse: transposes of chunk ci-2 done
                            sy.wait_ge(s_tp, 4 + 4 * (ci - 1))
                        sy.dma_start(
                            out=Traws[ci % 2][:, :],
                            in_=t_d[:, ci * 4096:(ci + 1) * 4096],
                        ).then_inc(s_tc[ci % 2], 16)

                @block.tensor
                def _(pe):
                    # inward transposes: [40,128] f32 tiles -> [128,40] in psT
                    # groups of 8 tiles; psT bank = grp % 4
                    def inw(grp, src_ap_fn):
                        if grp >= 4:
                            pe.wait_ge(s_tpc, grp - 3)
                        for k in range(8):
                            ins = pe.transpose(
                                psT[:, (grp % 4) * 512 + k * D:
                                    (grp % 4) * 512 + (k + 1) * D],
                                src_ap_fn(k),
                                eye40[:, :],
                            )
                        ins.then_inc(s_tp)

                    pe.wait_ge(s_e40, 16)
                    pe.wait_ge(s_e128, 16)
                    pe.wait_ge(s_xs, 16)
                    for g in range(2):
                        inw(g, lambda k, g=g: Xraw[:, (g * 8 + k) * 128:
                                                   (g * 8 + k + 1) * 128])
                    pe.wait_ge(s_ts, 16)
                    for g in range(2):
                        inw(2 + g, lambda k, g=g: Tsraw[:, (g * 8 + k) * 128:
                                                        (g * 8 + k + 1) * 128])
                    for ci in range(4):
                        pe.wait_ge(s_tc[ci % 2], 16 * (ci // 2 + 1))
                        for j in range(4):
                            inw(4 + ci * 4 + j,
                                lambda k, j=j, ci=ci: Traws[ci % 2][
                                    :, (j * 8 + k) * 128:(j * 8 + k + 1) * 128])

                    # outward transposes: [128,40] f16 tiles -> [40,128] in ps16
                    def outw(slot, og, src3, base_tile):
                        if slot >= 4:
                            prev = emit_order[slot - 4]
                            eng = out_eng[prev]
                            pe.wait_ge(s_tnv if eng == "v" else s_tna,
                                       ocv_e[prev] if eng == "v" else oca_e[prev])
                        for k in range(8):
                            t = base_tile + k
                            ins = pe.transpose(
                                ps16[0:D, (slot % 4) * 1024 + k * 128:
                                     (slot % 4) * 1024 + (k + 1) * 128],
                                src3[:, t * D:(t + 1) * D],
                                eye128[:, :],
                            )
                        ins.then_inc(s_tno)

                    pe.wait_ge(s_stat, 3)      # Xct16 ready
                    for slot, og in enumerate(emit_order):
                        if og == 16:
                            pass
                        elif og == 17:
                            pass
                        elif og == 0:
                            pe.wait_ge(s_stat, 4)      # Tnt ready
                        if og >= 16:
                            outw(slot, og, Xct16[:, :], (og - 16) * 8)
                        else:
                            outw(slot, og, Tnt[:, :], og * 8)

                @block.vector
                def _(v):
                    v.memset(zeros[:, :], 0.0)
                    v.drain()
                    # inward copies psT bank -> transposed layouts
                    for g in range(20):
                        v.wait_ge(s_tp, g + 1)
                        if g < 2:
                            dest = Xt[:, g * 320:(g + 1) * 320]
                        elif g < 4:
                            dest = Tst[:, (g - 2) * 320:(g - 1) * 320]
                        else:
                            dest = Tt[:, (g - 4) * 320:(g - 3) * 320]
                        v.tensor_copy(dest, psT[:, (g % 4) * 512:(g % 4) * 512 + 320]
                                      ).then_inc(s_tpc)
                    # stats, pipelined per 32-tile slab as T chunks land
                    xmu_bc = xmub[:, :].unsqueeze(2).broadcast_to((128, NRB, D))

                    def slab_ops(ci):
                        s32 = slice(ci * 32, (ci + 1) * 32)
                        t3 = Tt3[:, s32, :]
                        c3 = Tc3[:, s32, :]
                        q3 = sq3[:, s32, :]
                        mbc = mub[:, s32].unsqueeze(2).broadcast_to((128, 32, D))
                        return [
                            lambda: (v.wait_ge(s_tpc, 4 + 4 * (ci + 1)),
                                     v.reduce_sum(muS[:, s32], t3, axis=AX.X)),
                            lambda: v.tensor_scalar_mul(mub[:, s32],
                                                        muS[:, s32], 1.0 / D),
                            lambda: v.tensor_sub(c3, t3, mbc),
                            lambda: v.tensor_mul(q3, c3, c3),
                            lambda: v.reduce_sum(n2S[:, s32], q3, axis=AX.X),
                        ]

                    xchain = [
                        lambda: (v.wait_ge(s_tpc, 4),
                                 v.reduce_sum(xmuS[:, :], Tst3, axis=AX.X)),
                        lambda: v.tensor_scalar_mul(xmub[:, :], xmuS[:, :],
                                                    1.0 / D),
                        lambda: v.tensor_sub(Xct3, Xt3, xmu_bc),
                        lambda: v.tensor_mul(xsq3, Xct3, Xct3),
                        lambda: v.reduce_sum(xn2S[:, :], xsq3, axis=AX.X),
                    ]
                    _interleave(v, xchain, slab_ops(0))
                    _interleave(v, slab_ops(1))
                    _interleave(v, slab_ops(2))
                    _interleave(v, slab_ops(3))
                    v.drain()
                    clip = [
                        lambda: v.tensor_scalar_max(n2c[:, :], n2S[:, :], 1e-16
                                                    ).then_inc(s_stat),  # -> 1
                        lambda: v.tensor_scalar_max(xn2c[:, :], xn2S[:, :],
                                                    1e-16).then_inc(s_stat),  # 2
                    ]
                    _interleave(v, clip)
                    # rsqrt refine (needs ACT sqrt of n2c -> normb)
                    v.wait_ge(s_sqrt, 1)
                    r_bc = rTb[:, :].unsqueeze(2).broadcast_to((128, 128, D))
                    nrchain = [
                        lambda: v.reciprocal(y0b[:, :], normb[:, :]),
                        lambda: v.tensor_mul(t1b[:, :], y0b[:, :], y0b[:, :]),
                        lambda: v.tensor_mul(t2b[:, :], t1b[:, :], n2c[:, :]),
                        lambda: v.tensor_scalar(t3b[:, :], t2b[:, :], -0.5, 1.5,
                                                op0=OP.mult, op1=OP.add),
                        lambda: v.tensor_mul(rTb[:, :], y0b[:, :], t3b[:, :]),
                        lambda: v.tensor_mul(Tnt3, Tc3, r_bc
                                             ).then_inc(s_stat),   # -> 4
                    ]
                    othchain = [
                        lambda: v.tensor_scalar_mul(epsnx[:, :], xnormb[:, :],
                                                    EPS),
                        lambda: v.tensor_copy(Xct16[:, :], Xct[:, :]
                                              ).then_inc(s_stat),  # -> 3
                    ]
                    _interleave(v, nrchain, othchain)
                    # outward copies (vector share)
                    for slot, og in enumerate(emit_order):
                        if out_eng[og] != "v":
                            continue
                        v.wait_ge(s_tno, slot + 1)
                        src = ps16[0:D, (slot % 4) * 1024:(slot % 4 + 1) * 1024]
                        if og < 16:
                            v.tensor_copy(Tn[:, og * 1024:(og + 1) * 1024], src
                                          ).then_inc(s_tnv)
                        else:
                            v.tensor_copy(Xs[:, (og - 16) * 1024:(og - 15) * 1024],
                                          src).then_inc(s_tnv)

                @block.scalar
                def _(sc):
                    sc.wait_ge(s_stat, 2)
                    sc.activation(normb[:, :], n2c[:, :], AF.Sqrt)
                    sc.activation(xnormb[:, :], xn2c[:, :], AF.Sqrt
                                  ).then_inc(s_sqrt)
                    for slot, og in enumerate(emit_order):
                        if out_eng[og] != "a":
                            continue
                        sc.wait_ge(s_tno, slot + 1)
                        src = ps16[0:D, (slot % 4) * 1024:(slot % 4 + 1) * 1024]
                        sc.copy(Tn[:, og * 1024:(og + 1) * 1024], src
                                ).then_inc(s_tna)

            if NB == 0:
                # bisect stage "pre": preprocessing only
                @block.sync
                def _(sy):
                    sy.wait_ge(s_tnv, TOT_V)
                    sy.wait_ge(s_tna, TOT_A)
                    sy.dma_start(out=out_d[:, :], in_=epsnx[0:1, 0:1]
                                 ).then_inc(s_dma, 16)

            # ================= PHASE 2: main loop =================
            if NB > 0:
              with ExitStack() as p2:
                sb2 = lambda name, shape, dt: p2.enter_context(
                    nc.sbuf_tensor(name, shape, dt))
                S0 = sb2("S0", [128, N], F16)
                S1 = sb2("S1", [128, N], F16)
                wbuf = sb2("wbuf", [128, N], F16)
                TA = sb2("TA", [128, 8192], F16)
                TB = sb2("TB", [128, 4096], F16)
                TAm = sb2("TAm", [128, 5120], F16)
                TBm = sb2("TBm", [128, 2560], F16)
                psB = p2.enter_context(nc.psum_tensor("psB", [128, 4096], F32))
                Sb = [S0, S1]

                @block.tensor
                def _(pe):
                    pe.wait_ge(s_tnv, ocv_e[1])   # Xs + Tn cols [0, 4096)
                    if oca_e[1]:
                        pe.wait_ge(s_tna, oca_e[1])
                    for n in range(NB):
                        for g in range(NG):
                            K = n * NG + g
                            if K == 1:
                                # banks 4-7 were phase-1 ps16; also gates the
                                # rest of Tn
                                pe.wait_ge(s_tnv, TOT_V)
                                pe.wait_ge(s_tna, TOT_A)
                            if K >= 2:
                                prev = K - 2
                                if ext_is_act(prev % NG):
                                    pe.wait_ge(s_ea, act_cum[prev])
                                else:
                                    pe.wait_ge(s_ev, dve_cum[prev])
                            for c in range(4):
                                q = g * 4 + c
                                col = q * 512
                                ins = pe.matmul(
                                    psB[:, (q % 8) * 512:(q % 8 + 1) * 512],
                                    Xs[:, n * 128:(n + 1) * 128],
                                    Tn[:, col:col + 512],
                                )
                            # one inc per 2048-wide group: keeps the sem-inc
                            # rate low (cayman event-accel deadlock errata)
                            ins.then_inc(s_mm)

                def trees_and_b_thunks(v, n):
                    """row-min/max + temperature b for row block n (on DVE).

                    Chain thunks: consecutive entries depend on each other,
                    so they must be interleaved with an unrelated list."""
                    S = Sb[n % 2]
                    par = n % 2

                    def start():
                        v.wait_ge(s_ea, NGA * (n + 1))
                        v.wait_ge(s_ev, len(GD) * (n + 1))

                    tmin = _tree_thunks(v, S[:, 0:NGA * GRP], NGA * GRP, OP.min,
                                        mtree[:, :], TAm, TBm)
                    tmax = _tree_thunks(v, S[:, 0:N], N, OP.max,
                                        Mall[:, n:n + 1], TA, TB)
                    tmin[0] = (lambda f=tmin[0]: (start(), f()))
                    chain = [
                        lambda: v.tensor_tensor(mc1[:, :], minc[:, 0:1],
                                                minc[:, 1:2], op=OP.min),
                        lambda: v.tensor_tensor(mc2[:, :], mc1[:, :],
                                                minc[:, 2:3], op=OP.min),
                        lambda: v.tensor_tensor(mall[:, n:n + 1], mtree[:, :],
                                                mc2[:, :], op=OP.min),
                        lambda: v.tensor_add(mp[:, :], mall[:, n:n + 1],
                                             epsnx[:, n:n + 1]),
                        lambda: v.tensor_scalar_mul(dd[:, :], mp[:, :], -H),
                        lambda: v.reciprocal(ball[:, n:n + 1], dd[:, :]),
                        lambda: v.tensor_mul(Upar[par][:, 0:1],
                                             ball[:, n:n + 1], Mall[:, n:n + 1]),
                        lambda: v.tensor_mul(Upar[par][:, 1:2],
                                             ball[:, n:n + 1], mall[:, n:n + 1]
                                             ).then_inc(s_b),
                    ]
                    # tmin feeds chain[0]; keep them in one serial list
                    return tmin + chain, tmax

                def r_block_thunks(v, n):
                    """r_n = max(exp(bM), exp(bm)) / Z for row block n (DVE)."""
                    par = n % 2
                    return [
                        lambda: (v.wait_ge(s_z, n + 1),
                                 v.reciprocal(zr[:, :], Zall[:, n:n + 1])),
                        lambda: v.tensor_tensor(wm[:, :], Epar[par][:, 0:1],
                                                Epar[par][:, 1:2], op=OP.max),
                        lambda: v.tensor_mul(r_all[:, n:n + 1], wm[:, :],
                                             zr[:, :]),
                    ]

                @block.vector
                def _(v):
                    for n in range(NB):
                        if n >= 1:
                            a, b = trees_and_b_thunks(v, n - 1)
                            if n >= 2:
                                b = b + r_block_thunks(v, n - 2)
                            _interleave(v, a, b)
                        for j, g in enumerate(GD):
                            v.wait_ge(s_mm, n * NG + g + 1)
                            v.tensor_scalar(
                                out=Sb[n % 2][:, g * GRP:(g + 1) * GRP],
                                in0=psB[:, (g % 2) * GRP:(g % 2) * GRP + GRP],
                                scalar1=0.0,
                                scalar2=None,
                                op0=OP.add,
                                op1=OP.min,
                                accum_out=minc[:, j:j + 1],
                            ).then_inc(s_ev)
                    a, b = trees_and_b_thunks(v, NB - 1)
                    tail = r_block_thunks(v, NB - 2) if NB >= 2 else []
                    _interleave(v, a, b + tail)
                    _interleave(v, r_block_thunks(v, NB - 1))
                    v.drain()
                    v.reduce_max(rmaxb[:, :], r_all[:, 0:NB], axis=AX.X
                                 ).then_inc(s_r)
                    if stage == "full":
                        # cross-partition max via DMA round-trip transpose
                        v.wait_ge(s_cdma, 32)
                        v.reduce_max(gmx[:, :], rrow[0:1, :], axis=AX.X
                                     ).then_inc(s_fin)   # -> 1
                        v.wait_ge(s_fin, 2)
                        v.tensor_scalar_mul(outsb[:, :], lnb[:, :], -1.0
                                            ).then_inc(s_fin)   # -> 3

                def exp_block(sc, n):
                    sc.wait_ge(s_b, n + 1)
                    sc.wait_ge(s_ea, NGA * (n + 1))
                    sc.wait_ge(s_ev, len(GD) * (n + 1))
                    if n >= 1:   # certify WAW on wbuf/Epar vs prior blocks
                        sc.wait_ge(s_z, n)
                    par = n % 2
                    sc.activation(
                        wbuf[:, :],
                        Sb[par][:, :],
                        AF.Exp,
                        scale=ball[:, n:n + 1],
                        accum_out=Zall[:, n:n + 1],
                    )
                    sc.activation(Epar[par][:, :], Upar[par][:, :], AF.Exp
                                  ).then_inc(s_z)

                @block.scalar
                def _(sc):
                    for n in range(NB):
                        if n >= 2:
                            # S[n%2] was read by exp(n-2); certify those reads
                            # (small exp of block n-2 incs s_z to n-1)
                            sc.wait_ge(s_z, n - 1)
                        for g in range(NGA):
                            sc.wait_ge(s_mm, n * NG + g + 1)
                            sc.copy(Sb[n % 2][:, g * GRP:(g + 1) * GRP],
                                    psB[:, (g % 2) * GRP:(g % 2) * GRP + GRP]
                                    ).then_inc(s_ea)
                        if n >= 1:
                            exp_block(sc, n - 1)
                    exp_block(sc, NB - 1)
                    if stage == "full":
                        # finale: ln
                        sc.wait_ge(s_fin, 1)
                        sc.activation(lnb[:, :], gmx[:, :], AF.Ln
                                      ).then_inc(s_fin)   # -> 2

                @block.sync
                def _(sy):
                    if stage == "full":
                        sy.wait_ge(s_r, 1)
                        sy.dma_start(out=scr_d[:, :], in_=rmaxb[:, :]
                                     ).then_inc(s_cdma, 16)
                        sy.wait_ge(s_cdma, 16)
                        sy.dma_start(out=rrow[0:1, :],
                                     in_=scr_d[:, :].rearrange("a b -> b a")
                                     ).then_inc(s_cdma, 16)   # -> 32
                        sy.wait_ge(s_fin, 3)
                        sy.dma_start(out=out_d[:, :], in_=outsb[:, :]
                                     ).then_inc(s_dma, 16)
                    else:
                        sy.wait_ge(s_r, 1)
                        sy.dma_start(out=out_d[:, :], in_=rmaxb[0:1, 0:1]
                                     ).then_inc(s_dma, 16)

    # populate .instr bytes for extended-ISA instructions (TTR,
    # partition_all_reduce) — raw Bass skips this pass otherwise and the
    # NEFF compiler fails with "ISA wrong length"
    from concourse.library_overlay import lower_extended_insts
    lower_extended_insts(nc)
    return nc


_NC = None


def _get_nc():
    global _NC
    if _NC is None:
        _NC = build()
    return _NC


LAST_RESULT = None


def kernel(input, target_features, **bench_kwargs):
    global LAST_RESULT
    from concourse.bass_utils import run_bass_kernel_spmd

    X = np.ascontiguousarray(
        np.asarray(input, dtype=np.float32).reshape(50, N)[10:50])
    T = np.ascontiguousarray(
        np.asarray(target_features, dtype=np.float32).reshape(50, N)[10:50])
    in_maps = []
    for r in range(P):
        in_maps.append({
            "xs": np.ascontiguousarray(X[:, r * SH:(r + 1) * SH]),
            "ts": np.ascontiguousarray(T[:, r * SH:(r + 1) * SH]),
            "t": T,
        })
    nc = _get_nc()
    res = run_bass_kernel_spmd(nc, in_maps, core_ids=list(range(P)),
                               **bench_kwargs)
    LAST_RESULT = res
    # each core returns -log(max CX over its row shard); the global
    # -log(max) is the min across cores (log is monotone)
    vals = [np.asarray(res.results[r]["out"], dtype=np.float32).reshape(())
            for r in range(P)]
    return np.min(np.stack(vals)).astype(np.float32).reshape(())


# revision 44
# speedup vs baseline: 1.2544x; 1.0004x over previous
"""Trainium2 Bass kernel for nn_ContextualLoss (8 NeuronCores, SPMD).

Math (derived from the reference):
  X = vec(input)[10:50] - mu,  T = vec(target)[10:50] - mu,  mu = colmean(target)
  S[i,j] = cos(x_i, t_j);  CX = softmax_j(a_i * S[i,j]),  a_i = -1/(h*(min_j S + eps))
  loss = -log(max_ij CX)
X's column normalization folds into the softmax temperature:
  logits = b_i * R[i,j],  R = Xc^T Tn  (Xc centered only, Tn column-normalized)
  b_i = -1/(h*(min_j R + eps*||x_i||))
Per row we need: min_j R (f32/fp16), max_j R (fp16), sum_j exp(b_i R) (f32 acc).
Final: per-core max of exp(b*M)/Z, AllReduce-max over 8 cores, -log.

Sharding: each core computes 2048 of the 16384 S-rows (x-columns).
"""

import numpy as np
from contextlib import ExitStack

import concourse.bass as bass
import concourse.mybir as mybir
from concourse import bass_isa

F32 = mybir.dt.float32
F16 = mybir.dt.float16
AF = mybir.ActivationFunctionType
OP = mybir.AluOpType
AX = mybir.AxisListType

D = 40          # contraction dim (rows 10:50)
N = 16384       # feature columns
P = 8           # cores
SH = N // P     # x-columns per core
NRB = SH // 128  # row blocks per core = 16
NG = 8          # 2048-wide column groups per row block
NGA = 5         # groups extracted by ScalarE (plain copy)
GD = [5, 6, 7]  # groups extracted by VectorE (TTR copy+min)
GRP = 2048      # group width
EPS = 1e-5
H = 0.2
BIGF = 3.0e38


def _tree_thunks(v, src, width, op, out_slice, TA, TB, stop=512):
    """Thunks for a pairwise-halving fp16 reduce on VectorE.

    Returned thunks must be interleaved with >=1 unrelated op between
    consecutive ones (DVE pipeline output hazard: an op may not read the
    immediately-preceding op's output without a drain)."""
    thunks = []
    cur, w = src, width
    bufs = [TA, TB]
    bi = 0
    while w > stop:
        h = w // 2
        dst = bufs[bi]
        thunks.append(lambda cur=cur, h=h, w=w, dst=dst:
                      v.tensor_tensor(dst[:, 0:h], cur[:, 0:h], cur[:, h:w],
                                      op=op))
        cur, w = bufs[bi], h
        bi ^= 1
    thunks.append(lambda cur=cur, w=w:
                  v.tensor_reduce(out_slice, cur[:, 0:w], axis=AX.X, op=op))
    return thunks


def _interleave(v, *lists):
    """Emit ops round-robin from the lists with a drain between rounds.

    The race model requires every same-engine RAW/WAW pair to be separated
    by a drain (or a sem-certified wait); ops within one round are mutually
    independent, so one drain per round suffices and overlaps round issue."""
    n = max(len(l) for l in lists)
    for i in range(n):
        for l in lists:
            if i < len(l):
                l[i]()
        if i < n - 1:
            v.drain()


def build():
    import os
    stage = os.environ.get("K_STAGE", "full")   # debug bisect hook
    if stage == "pre":
        NB = 0
    elif stage.startswith("main"):
        NB = int(stage[4:])
    else:
        NB = NRB

    nc = bass.Bass(num_devices=P)

    xs_d = nc.declare_dram_parameter("xs", [D, SH], F32, isOutput=False)
    ts_d = nc.declare_dram_parameter("ts", [D, SH], F32, isOutput=False)
    t_d = nc.declare_dram_parameter("t", [D, N], F32, isOutput=False)
    out_d = nc.declare_dram_parameter("out", [1, 1], F32, isOutput=True)

    eye40_d = nc.inline_tensor(np.eye(D, dtype=np.float32), "eye40c")
    eye128_d = nc.inline_tensor(np.eye(128, dtype=np.float16), "eye128c")
    scr_d = nc.dram_tensor("scrmax", [128, 1], F32)

    ctx = ExitStack()
    with ctx:
        sbuf = lambda name, shape, dt: ctx.enter_context(
            nc.sbuf_tensor(name, shape, dt))
        sem = lambda name: ctx.enter_context(nc.semaphore(name))

        # ---- persistent SBUF ----
        eye40 = sbuf("eye40", [D, D], F32)
        eye128 = sbuf("eye128", [128, 128], F16)
        Tn = sbuf("Tn", [D, N], F16)         # normalized target, matmul rhs
        Xs = sbuf("Xsb", [D, SH], F16)       # centered x shard, matmul lhsT
        zeros = sbuf("zeros", [128, GRP], F32)
        epsnx = sbuf("epsnx", [128, NRB], F32)
        Mall = sbuf("Mall", [128, NRB], F32)
        mall = sbuf("mall", [128, NRB], F32)
        ball = sbuf("ball", [128, NRB], F32)
        Zall = sbuf("Zall", [128, NRB], F32)
        r_all = sbuf("r_all", [128, NRB], F32)
        minc = sbuf("minc", [128, len(GD)], F32)
        Upar = [sbuf(f"U{i}", [128, 2], F32) for i in range(2)]
        Epar = [sbuf(f"E{i}", [128, 2], F32) for i in range(2)]
        Zpar = [sbuf(f"Zp{i}", [128, 4], F32) for i in range(2)]
        mtree = sbuf("mtree", [128, 1], F32)
        mc1 = sbuf("mc1", [128, 1], F32)
        mc2 = sbuf("mc2", [128, 1], F32)
        mp = sbuf("mp", [128, 1], F32)
        dd = sbuf("dd", [128, 1], F32)
        zr = sbuf("zr", [128, 1], F32)
        wm = sbuf("wm", [128, 1], F32)
        rmaxb = sbuf("rmaxb", [128, 1], F32)
        rrow = sbuf("rrow", [1, 128], F32)
        gmx = sbuf("gmx", [1, 1], F32)
        prb = sbuf("prb", [128, 1], F32)
        gmb = sbuf("gmb", [1, 1], F32)
        lnb = sbuf("lnb", [1, 1], F32)
        outsb = sbuf("outsb", [1, 1], F32)

        s_dma = sem("s_dma")
        s_e40 = sem("s_e40")
        s_e128 = sem("s_e128")
        s_xs = sem("s_xs")
        s_ts = sem("s_ts")
        s_tc = [sem("s_tc0"), sem("s_tc1")]
        s_tp = sem("s_tp")
        s_tpc = sem("s_tpc")
        s_stat = sem("s_stat")
        s_sqrt = sem("s_sqrt")
        s_tno = sem("s_tno")
        s_tnv = sem("s_tnv")
        s_tna = sem("s_tna")
        s_mm = sem("s_mm")
        s_ev = sem("s_ev")
        s_ea = sem("s_ea")
        s_b = sem("s_b")
        s_z = sem("s_z")
        s_r = sem("s_r")
        s_fin = sem("s_fin")
        s_cdma = sem("s_cdma")

        # outward-copy engine assignment: og 0..15 = Tn groups, 16..17 = Xs
        out_eng = ["v" if (og % 8) < 5 else "a" for og in range(16)] + ["v", "v"]
        # emission order: X groups first so the main loop can start early
        emit_order = [16, 17] + list(range(16))
        # cumulative per-engine outward-copy counts in EMISSION order, by og
        ocv_e, oca_e = {}, {}
        cv = ca = 0
        for og in emit_order:
            if out_eng[og] == "v":
                cv += 1
            else:
                ca += 1
            ocv_e[og] = cv
            oca_e[og] = ca
        TOT_V, TOT_A = cv, ca

        # main-loop extraction: DVE takes kdve(n) groups (fused copy+min),
        # ScalarE copies the rest; alternate 3/2 to balance the engines
        def kdve(n):
            return 3 if n % 2 == 0 else 2

        def nga(n):
            return NG - kdve(n)

        def ext_is_act(n, g):
            return g < nga(n)

        # cumulative extraction counts after global group K
        act_cum, dve_cum = [], []
        a = v = 0
        for K in range(NRB * NG):
            if ext_is_act(K // NG, K % NG):
                a += 1
            else:
                v += 1
            act_cum.append(a)
            dve_cum.append(v)

        with nc.Block() as block:
            # ================= PHASE 1: load + preprocess =================
            with ExitStack() as p1:
                sb1 = lambda name, shape, dt: p1.enter_context(
                    nc.sbuf_tensor(name, shape, dt))
                Traw0 = sb1("Traw0", [D, 4096], F32)
                Traw1 = sb1("Traw1", [D, 4096], F32)
                Xraw = sb1("Xraw", [D, SH], F32)
                Tsraw = sb1("Tsraw", [D, SH], F32)
                Tt = sb1("Tt", [128, 5120], F32)
                Tc = sb1("Tc", [128, 5120], F32)
                sqb = sb1("sqb", [128, 5120], F32)
                Tnt = sb1("Tnt", [128, 5120], F16)
                Xt = sb1("Xt", [128, 640], F32)
                Xct = sb1("Xct", [128, 640], F32)
                xsqb = sb1("xsqb", [128, 640], F32)
                Xct16 = sb1("Xct16", [128, 640], F16)
                Tst = sb1("Tst", [128, 640], F32)
                muS = sb1("muS", [128, 128], F32)
                mub = sb1("mub", [128, 128], F32)
                n2S = sb1("n2S", [128, 128], F32)
                n2c = sb1("n2c", [128, 128], F32)
                normb = sb1("normb", [128, 128], F32)
                y0b = sb1("y0b", [128, 128], F32)
                t1b = sb1("t1b", [128, 128], F32)
                t2b = sb1("t2b", [128, 128], F32)
                t3b = sb1("t3b", [128, 128], F32)
                rTb = sb1("rTb", [128, 128], F32)
                xmuS = sb1("xmuS", [128, NRB], F32)
                xmub = sb1("xmub", [128, NRB], F32)
                xn2S = sb1("xn2S", [128, NRB], F32)
                xn2c = sb1("xn2c", [128, NRB], F32)
                xnormb = sb1("xnormb", [128, NRB], F32)
                psT = p1.enter_context(nc.psum_tensor("psT", [128, 2048], F32))
                ps16 = p1.enter_context(nc.psum_tensor("ps16", [128, 4096], F16))
                Traws = [Traw0, Traw1]
                Tt3 = Tt[:, :].rearrange("p (t d) -> p t d", d=D)
                Tc3 = Tc[:, :].rearrange("p (t d) -> p t d", d=D)
                sq3 = sqb[:, :].rearrange("p (t d) -> p t d", d=D)
                Tnt3 = Tnt[:, :].rearrange("p (t d) -> p t d", d=D)
                Xt3 = Xt[:, :].rearrange("p (t d) -> p t d", d=D)
                Xct3 = Xct[:, :].rearrange("p (t d) -> p t d", d=D)
                xsq3 = xsqb[:, :].rearrange("p (t d) -> p t d", d=D)
                Tst3 = Tst[:, :].rearrange("p (t d) -> p t d", d=D)

                @block.sync
                def _(sy):
                    sy.dma_start(out=eye40[:, :], in_=eye40_d[:, :]).then_inc(s_e40, 16)
                    sy.dma_start(out=eye128[:, :], in_=eye128_d[:, :]).then_inc(s_e128, 16)
                    sy.dma_start(out=Xraw[:, :], in_=xs_d[:, :]).then_inc(s_xs, 16)
                    sy.dma_start(out=Tsraw[:, :], in_=ts_d[:, :]).then_inc(s_ts, 16)
                    for ci in range(4):
                        if ci >= 2:
                            # buffer reuse: transposes of chunk ci-2 done
                            sy.wait_ge(s_tp, 4 + 4 * (ci - 1))
                        sy.dma_start(
                            out=Traws[ci % 2][:, :],
                            in_=t_d[:, ci * 4096:(ci + 1) * 4096],
                        ).then_inc(s_tc[ci % 2], 16)

                @block.tensor
                def _(pe):
                    # inward transposes: [40,128] f32 tiles -> [128,40] in psT
                    # groups of 8 tiles; psT bank = grp % 4
                    def inw(grp, src_ap_fn):
                        if grp >= 4:
                            pe.wait_ge(s_tpc, grp - 3)
                        for k in range(8):
                            ins = pe.transpose(
                                psT[:, (grp % 4) * 512 + k * D:
                                    (grp % 4) * 512 + (k + 1) * D],
                                src_ap_fn(k),
                                eye40[:, :],
                            )
                        ins.then_inc(s_tp)

                    pe.wait_ge(s_e40, 16)
                    pe.wait_ge(s_e128, 16)
                    pe.wait_ge(s_xs, 16)
                    for g in range(2):
                        inw(g, lambda k, g=g: Xraw[:, (g * 8 + k) * 128:
                                                   (g * 8 + k + 1) * 128])
                    pe.wait_ge(s_ts, 16)
                    for g in range(2):
                        inw(2 + g, lambda k, g=g: Tsraw[:, (g * 8 + k) * 128:
                                                        (g * 8 + k + 1) * 128])
                    for ci in range(4):
                        pe.wait_ge(s_tc[ci % 2], 16 * (ci // 2 + 1))
                        for j in range(4):
                            inw(4 + ci * 4 + j,
                                lambda k, j=j, ci=ci: Traws[ci % 2][
                                    :, (j * 8 + k) * 128:(j * 8 + k + 1) * 128])

                    # outward transposes: [128,40] f16 tiles -> [40,128] in ps16
                    def outw(slot, og, src3, base_tile):
                        if slot >= 4:
                            prev = emit_order[slot - 4]
                            eng = out_eng[prev]
                            pe.wait_ge(s_tnv if eng == "v" else s_tna,
                                       ocv_e[prev] if eng == "v" else oca_e[prev])
                        for k in range(8):
                            t = base_tile + k
                            ins = pe.transpose(
                                ps16[0:D, (slot % 4) * 1024 + k * 128:
                                     (slot % 4) * 1024 + (k + 1) * 128],
                                src3[:, t * D:(t + 1) * D],
                                eye128[:, :],
                            )
                        ins.then_inc(s_tno)

                    pe.wait_ge(s_stat, 3)      # Xct16 ready
                    for slot, og in enumerate(emit_order):
                        if og == 16:
                            pass
                        elif og == 17:
                            pass
                        elif og == 0:
                            pe.wait_ge(s_stat, 4)      # Tnt ready
                        if og >= 16:
                            outw(slot, og, Xct16[:, :], (og - 16) * 8)
                        else:
                            outw(slot, og, Tnt[:, :], og * 8)

                @block.vector
                def _(v):
                    v.memset(zeros[:, :], 0.0)
                    v.drain()
                    # inward copies psT bank -> transposed layouts
                    for g in range(20):
                        v.wait_ge(s_tp, g + 1)
                        if g < 2:
                            dest = Xt[:, g * 320:(g + 1) * 320]
                        elif g < 4:
                            dest = Tst[:, (g - 2) * 320:(g - 1) * 320]
                        else:
                            dest = Tt[:, (g - 4) * 320:(g - 3) * 320]
                        v.tensor_copy(dest, psT[:, (g % 4) * 512:(g % 4) * 512 + 320]
                                      ).then_inc(s_tpc)
                    # stats, pipelined per 32-tile slab as T chunks land
                    xmu_bc = xmub[:, :].unsqueeze(2).broadcast_to((128, NRB, D))

                    def slab_ops(ci):
                        s32 = slice(ci * 32, (ci + 1) * 32)
                        t3 = Tt3[:, s32, :]
                        c3 = Tc3[:, s32, :]
                        q3 = sq3[:, s32, :]
                        mbc = mub[:, s32].unsqueeze(2).broadcast_to((128, 32, D))
                        return [
                            lambda: (v.wait_ge(s_tpc, 4 + 4 * (ci + 1)),
                                     v.reduce_sum(muS[:, s32], t3, axis=AX.X)),
                            lambda: v.tensor_scalar_mul(mub[:, s32],
                                                        muS[:, s32], 1.0 / D),
                            lambda: v.tensor_sub(c3, t3, mbc),
                            lambda: v.tensor_mul(q3, c3, c3),
                            lambda: v.reduce_sum(n2S[:, s32], q3, axis=AX.X),
                        ]

                    xchain = [
                        lambda: (v.wait_ge(s_tpc, 4),
                                 v.reduce_sum(xmuS[:, :], Tst3, axis=AX.X)),
                        lambda: v.tensor_scalar_mul(xmub[:, :], xmuS[:, :],
                                                    1.0 / D),
                        lambda: v.tensor_sub(Xct3, Xt3, xmu_bc),
                        lambda: v.tensor_mul(xsq3, Xct3, Xct3),
                        lambda: v.reduce_sum(xn2S[:, :], xsq3, axis=AX.X),
                    ]
                    _interleave(v, xchain, slab_ops(0))
                    _interleave(v, slab_ops(1))
                    _interleave(v, slab_ops(2))
                    _interleave(v, slab_ops(3))
                    v.drain()
                    clip = [
                        lambda: v.tensor_scalar_max(n2c[:, :], n2S[:, :], 1e-16
                                                    ).then_inc(s_stat),  # -> 1
                        lambda: v.tensor_scalar_max(xn2c[:, :], xn2S[:, :],
                                                    1e-16).then_inc(s_stat),  # 2
                    ]
                    _interleave(v, clip)
                    # rsqrt refine (needs ACT sqrt of n2c -> normb)
                    v.wait_ge(s_sqrt, 1)
                    r_bc = rTb[:, :].unsqueeze(2).broadcast_to((128, 128, D))
                    nrchain = [
                        lambda: v.reciprocal(y0b[:, :], normb[:, :]),
                        lambda: v.tensor_mul(t1b[:, :], y0b[:, :], y0b[:, :]),
                        lambda: v.tensor_mul(t2b[:, :], t1b[:, :], n2c[:, :]),
                        lambda: v.tensor_scalar(t3b[:, :], t2b[:, :], -0.5, 1.5,
                                                op0=OP.mult, op1=OP.add),
                        lambda: v.tensor_mul(rTb[:, :], y0b[:, :], t3b[:, :]),
                        lambda: v.tensor_mul(Tnt3, Tc3, r_bc
                                             ).then_inc(s_stat),   # -> 4
                    ]
                    othchain = [
                        lambda: v.tensor_scalar_mul(epsnx[:, :], xnormb[:, :],
                                                    EPS),
                        lambda: v.tensor_copy(Xct16[:, :], Xct[:, :]
                                              ).then_inc(s_stat),  # -> 3
                    ]
                    _interleave(v, nrchain, othchain)
                    # outward copies (vector share)
                    for slot, og in enumerate(emit_order):
                        if out_eng[og] != "v":
                            continue
                        v.wait_ge(s_tno, slot + 1)
                        src = ps16[0:D, (slot % 4) * 1024:(slot % 4 + 1) * 1024]
                        if og < 16:
                            v.tensor_copy(Tn[:, og * 1024:(og + 1) * 1024], src
                                          ).then_inc(s_tnv)
                        else:
                            v.tensor_copy(Xs[:, (og - 16) * 1024:(og - 15) * 1024],
                                          src).then_inc(s_tnv)

                @block.scalar
                def _(sc):
                    sc.wait_ge(s_stat, 2)
                    sc.activation(normb[:, :], n2c[:, :], AF.Sqrt)
                    sc.activation(xnormb[:, :], xn2c[:, :], AF.Sqrt
                                  ).then_inc(s_sqrt)
                    for slot, og in enumerate(emit_order):
                        if out_eng[og] != "a":
                            continue
                        sc.wait_ge(s_tno, slot + 1)
                        src = ps16[0:D, (slot % 4) * 1024:(slot % 4 + 1) * 1024]
                        sc.copy(Tn[:, og * 1024:(og + 1) * 1024], src
                                ).then_inc(s_tna)

            if NB == 0:
                # bisect stage "pre": preprocessing only
                @block.sync
                def _(sy):
                    sy.wait_ge(s_tnv, TOT_V)
                    sy.wait_ge(s_tna, TOT_A)
                    sy.dma_start(out=out_d[:, :], in_=epsnx[0:1, 0:1]
                                 ).then_inc(s_dma, 16)

            # ================= PHASE 2: main loop =================
            if NB > 0:
              with ExitStack() as p2:
                sb2 = lambda name, shape, dt: p2.enter_context(
                    nc.sbuf_tensor(name, shape, dt))
                S0 = sb2("S0", [128, N], F16)
                S1 = sb2("S1", [128, N], F16)
                wbuf = sb2("wbuf", [128, N], F16)
                TA = sb2("TA", [128, 8192], F16)
                TB = sb2("TB", [128, 4096], F16)
                TAm = sb2("TAm", [128, 6144], F16)
                TBm = sb2("TBm", [128, 3072], F16)
                psB = p2.enter_context(nc.psum_tensor("psB", [128, 4096], F32))
                Sb = [S0, S1]

                @block.tensor
                def _(pe):
                    pe.wait_ge(s_tnv, ocv_e[1])   # Xs + Tn cols [0, 4096)
                    if oca_e[1]:
                        pe.wait_ge(s_tna, oca_e[1])
                    for n in range(NB):
                        for g in range(NG):
                            K = n * NG + g
                            if K == 1:
                                # banks 4-7 were phase-1 ps16; also gates the
                                # rest of Tn
                                pe.wait_ge(s_tnv, TOT_V)
                                pe.wait_ge(s_tna, TOT_A)
                            if K >= 2:
                                prev = K - 2
                                if ext_is_act(prev // NG, prev % NG):
                                    pe.wait_ge(s_ea, act_cum[prev])
                                else:
                                    pe.wait_ge(s_ev, dve_cum[prev])
                            for c in range(4):
                                q = g * 4 + c
                                col = q * 512
                                ins = pe.matmul(
                                    psB[:, (q % 8) * 512:(q % 8 + 1) * 512],
                                    Xs[:, n * 128:(n + 1) * 128],
                                    Tn[:, col:col + 512],
                                )
                            # one inc per 2048-wide group: keeps the sem-inc
                            # rate low (cayman event-accel deadlock errata)
                            ins.then_inc(s_mm)

                def trees_and_b_thunks(v, n):
                    """row-min/max + temperature b for row block n (on DVE).

                    Chain thunks: consecutive entries depend on each other,
                    so they must be interleaved with an unrelated list."""
                    S = Sb[n % 2]
                    par = n % 2

                    def start():
                        v.wait_ge(s_ea, act_cum[n * NG + NG - 1])
                        v.wait_ge(s_ev, dve_cum[n * NG + NG - 1])

                    wmin = nga(n) * GRP
                    tmin = _tree_thunks(v, S[:, 0:wmin], wmin, OP.min,
                                        mtree[:, :], TAm, TBm)
                    tmax = _tree_thunks(v, S[:, 0:N], N, OP.max,
                                        Mall[:, n:n + 1], TA, TB)
                    tmin[0] = (lambda f=tmin[0]: (start(), f()))
                    if kdve(n) == 3:
                        comb = [
                            lambda: v.tensor_tensor(mc1[:, :], minc[:, 0:1],
                                                    minc[:, 1:2], op=OP.min),
                            lambda: v.tensor_tensor(mc2[:, :], mc1[:, :],
                                                    minc[:, 2:3], op=OP.min),
                        ]
                    else:
                        comb = [
                            lambda: v.tensor_tensor(mc2[:, :], minc[:, 0:1],
                                                    minc[:, 1:2], op=OP.min),
                        ]
                    chain = comb + [
                        lambda: v.tensor_tensor(mall[:, n:n + 1], mtree[:, :],
                                                mc2[:, :], op=OP.min),
                        lambda: v.tensor_add(mp[:, :], mall[:, n:n + 1],
                                             epsnx[:, n:n + 1]),
                        lambda: v.tensor_scalar_mul(dd[:, :], mp[:, :], -H),
                        lambda: v.reciprocal(ball[:, n:n + 1], dd[:, :]),
                        lambda: v.tensor_mul(Upar[par][:, 0:1],
                                             ball[:, n:n + 1], Mall[:, n:n + 1]),
                        lambda: v.tensor_mul(Upar[par][:, 1:2],
                                             ball[:, n:n + 1], mall[:, n:n + 1]
                                             ).then_inc(s_b),
                    ]
                    # tmin feeds chain[0]; keep them in one serial list
                    return tmin + chain, tmax

                def r_block_thunks(v, n):
                    """r_n = max(exp(bM), exp(bm)) / Z for row block n (DVE)."""
                    par = n % 2
                    return [
                        lambda: (v.wait_ge(s_z, n + 1),
                                 v.reciprocal(zr[:, :], Zall[:, n:n + 1])),
                        lambda: v.tensor_tensor(wm[:, :], Epar[par][:, 0:1],
                                                Epar[par][:, 1:2], op=OP.max),
                        lambda: v.tensor_mul(r_all[:, n:n + 1], wm[:, :],
                                             zr[:, :]),
                    ]

                @block.vector
                def _(v):
                    for n in range(NB):
                        if n >= 1:
                            a, b = trees_and_b_thunks(v, n - 1)
                            if n >= 2:
                                b = b + r_block_thunks(v, n - 2)
                            _interleave(v, a, b)
                        for j, g in enumerate(range(nga(n), NG)):
                            v.wait_ge(s_mm, n * NG + g + 1)
                            v.tensor_scalar(
                                out=Sb[n % 2][:, g * GRP:(g + 1) * GRP],
                                in0=psB[:, (g % 2) * GRP:(g % 2) * GRP + GRP],
                                scalar1=0.0,
                                scalar2=None,
                                op0=OP.add,
                                op1=OP.min,
                                accum_out=minc[:, j:j + 1],
                            ).then_inc(s_ev)
                    a, b = trees_and_b_thunks(v, NB - 1)
                    tail = r_block_thunks(v, NB - 2) if NB >= 2 else []
                    _interleave(v, a, b + tail)
                    _interleave(v, r_block_thunks(v, NB - 1))
                    v.drain()
                    v.reduce_max(rmaxb[:, :], r_all[:, 0:NB], axis=AX.X
                                 ).then_inc(s_r)
                    if stage == "full":
                        # cross-partition max via DMA round-trip transpose
                        v.wait_ge(s_cdma, 32)
                        v.reduce_max(gmx[:, :], rrow[0:1, :], axis=AX.X
                                     ).then_inc(s_fin)   # -> 1
                        v.wait_ge(s_fin, 2)
                        v.tensor_scalar_mul(outsb[:, :], lnb[:, :], -1.0
                                            ).then_inc(s_fin)   # -> 3

                def exp_block(sc, n):
                    sc.wait_ge(s_b, n + 1)
                    sc.wait_ge(s_ea, act_cum[n * NG + NG - 1])
                    sc.wait_ge(s_ev, dve_cum[n * NG + NG - 1])
                    if n >= 1:   # certify WAW on wbuf/Epar vs prior blocks
                        sc.wait_ge(s_z, n)
                    par = n % 2
                    sc.activation(
                        wbuf[:, :],
                        Sb[par][:, :],
                        AF.Exp,
                        scale=ball[:, n:n + 1],
                        accum_out=Zall[:, n:n + 1],
                    )
                    sc.activation(Epar[par][:, :], Upar[par][:, :], AF.Exp
                                  ).then_inc(s_z)

                @block.scalar
                def _(sc):
                    for n in range(NB):
                        if n >= 2:
                            # S[n%2] was read by exp(n-2); certify those reads
                            # (small exp of block n-2 incs s_z to n-1)
                            sc.wait_ge(s_z, n - 1)
                        for g in range(nga(n)):
                            sc.wait_ge(s_mm, n * NG + g + 1)
                            sc.copy(Sb[n % 2][:, g * GRP:(g + 1) * GRP],
                                    psB[:, (g % 2) * GRP:(g % 2) * GRP + GRP]
                                    ).then_inc(s_ea)
                        if n >= 1:
                            exp_block(sc, n - 1)
                    exp_block(sc, NB - 1)
                    if stage == "full":
                        # finale: ln
                        sc.wait_ge(s_fin, 1)
                        sc.activation(lnb[:, :], gmx[:, :], AF.Ln
                                      ).then_inc(s_fin)   # -> 2

                @block.sync
                def _(sy):
                    if stage == "full":
                        sy.wait_ge(s_r, 1)
                        sy.dma_start(out=scr_d[:, :], in_=rmaxb[:, :]
                                     ).then_inc(s_cdma, 16)
                        sy.wait_ge(s_cdma, 16)
                        sy.dma_start(out=rrow[0:1, :],
                                     in_=scr_d[:, :].rearrange("a b -> b a")
                                     ).then_inc(s_cdma, 16)   # -> 32
                        sy.wait_ge(s_fin, 3)
                        sy.dma_start(out=out_d[:, :], in_=outsb[:, :]
                                     ).then_inc(s_dma, 16)
                    else:
                        sy.wait_ge(s_r, 1)
                        sy.dma_start(out=out_d[:, :], in_=rmaxb[0:1, 0:1]
                                     ).then_inc(s_dma, 16)

    # populate .instr bytes for extended-ISA instructions (TTR,
    # partition_all_reduce) — raw Bass skips this pass otherwise and the
    # NEFF compiler fails with "ISA wrong length"
    from concourse.library_overlay import lower_extended_insts
    lower_extended_insts(nc)
    return nc


_NC = None


def _get_nc():
    global _NC
    if _NC is None:
        _NC = build()
    return _NC


LAST_RESULT = None


def kernel(input, target_features, **bench_kwargs):
    global LAST_RESULT
    from concourse.bass_utils import run_bass_kernel_spmd

    X = np.ascontiguousarray(
        np.asarray(input, dtype=np.float32).reshape(50, N)[10:50])
    T = np.ascontiguousarray(
        np.asarray(target_features, dtype=np.float32).reshape(50, N)[10:50])
    in_maps = []
    for r in range(P):
        in_maps.append({
            "xs": np.ascontiguousarray(X[:, r * SH:(r + 1) * SH]),
            "ts": np.ascontiguousarray(T[:, r * SH:(r + 1) * SH]),
            "t": T,
        })
    nc = _get_nc()
    res = run_bass_kernel_spmd(nc, in_maps, core_ids=list(range(P)),
                               **bench_kwargs)
    LAST_RESULT = res
    # each core returns -log(max CX over its row shard); the global
    # -log(max) is the min across cores (log is monotone)
    vals = [np.asarray(res.results[r]["out"], dtype=np.float32).reshape(())
            for r in range(P)]
    return np.min(np.stack(vals)).astype(np.float32).reshape(())
